# revision 27
# baseline (speedup 1.0000x reference)
"""Multi-head self-attention (B=4, L=2048, D=1024, H=16, RoPE, causal) on 8
Trainium2 NeuronCores.

Sharding: data-parallel over batch (4) x tensor-parallel over head groups (2).
Core i handles batch i//2, heads 8*(i%2) .. 8*(i%2)+8.  Each core computes its
QKV projection slice, RoPE, causal attention for its 8 heads, and a partial
output projection over its 512 d-columns; the host sums the two partials per
batch.

On-core dataflow (per core, matmul operands bf16, psum/softmax fp32):
  qkT[e,l] = Wqk_sub @ x^T       (e = 8 q-heads then 8 k-heads, dh-major)
  rope on qkT rows (pair-swap via stream_shuffle + bf16 cos/sin tables)
  V[l,e]   = x @ Wv_sub^T        stored with a ones column per head:
             vsb[k][:, 65h:65h+64] = V_h, vsb[k][:, 65h+64] = 1
  per head-pair p (heads 2p, 2p+1), per 512-wide q block:
    S^T[k,q] = K^T Q  (row-split pair, tile_position (0,0)/(64,0))
    P^T = exp(S^T/8), one contiguous ACT op + tri-mask per head
    av[0:64] / s[64] = [V_h | ones]^T @ P^T_h   (full-array, 65-col stationary
                       -> AV and the softmax denominator in one stream pass)
    norm: s rows staged to SBUF partition 64, 1/s in place on partition 64
    (DVE), broadcast across partitions via K=1 ones-matmuls at array row 64
    into PSUM, per-head muls on DVE
  y[l,e] partial = O^T.T @ Wo_sub  (lhsT = O^T chunks), bf16 output

Scheduling: demand-driven.  The straight-line head is only qk chunks c0/c4
l4=0 plus V0-3; every other qk chunk / V tile is woven into the attention
rounds as PE filler so the scalar engine (exp, the attention-phase pacer)
starts ~15us in and the PE never idles long enough for the HAM clock gate to
re-throttle mid-kernel.  Each round's normalization tail is deferred into the
next round.  Inputs are host-packed contiguous so every prologue DMA is one
descriptor per partition, ordered by first use.
"""
import sys
sys.path.insert(0, "/opt/trn_rl_repo")

import numpy as np
import ml_dtypes

B, L, D, H = 4, 2048, 1024, 16
DH = D // H  # 64
THETA = 100000.0
NCORES = 8
BF = ml_dtypes.bfloat16

_built = None


def _rope_tables():
    # [128, L] bf16: rows = 2 stacked heads' dh (64 each), identical per head.
    pos = np.arange(L, dtype=np.float32)
    inv_freq = (1.0 / THETA ** (np.arange(0, DH, 2, dtype=np.float32) / DH))
    ang = pos[None, :] * inv_freq[:, None]              # [32, L]
    cos = np.cos(ang)                                    # [32, L]
    sin = np.sin(ang)
    cos2 = np.repeat(cos, 2, axis=0)                     # rows 2p,2p+1 = cos_p
    sin2 = np.empty((DH, L), np.float32)
    sin2[0::2] = -sin
    sin2[1::2] = sin
    return (np.concatenate([cos2, cos2], 0).astype(BF),
            np.concatenate([sin2, sin2], 0).astype(BF))


def _build():
    import concourse.mybir as mybir
    import concourse.tile as tile
    from concourse import bacc

    FP32 = mybir.dt.float32
    BF16 = mybir.dt.bfloat16
    MUL = mybir.AluOpType.mult
    ADD = mybir.AluOpType.add
    EXP = mybir.ActivationFunctionType.Exp
    SWAP_MASK = [i ^ 1 for i in range(32)]

    nc = bacc.Bacc(None, target_bir_lowering=False)
    # host-packed layouts, fully contiguous per DMA
    xt_d = nc.declare_dram_parameter("xt", [4, 128, 4096], BF16, False)
    wqk_d = nc.declare_dram_parameter("wqk", [8, 128, 1024], BF16, False)
    wv_d = nc.declare_dram_parameter("wv", [128, 4096], BF16, False)
    wo_d = nc.declare_dram_parameter("wo", [128, 4096], BF16, False)
    cos_d = nc.declare_dram_parameter("cos2", [128, L], BF16, False)
    sin_d = nc.declare_dram_parameter("sin2", [128, L], BF16, False)
    tri_d = nc.declare_dram_parameter("trimask", [128, 128], BF16, False)
    y_d = nc.declare_dram_parameter("y", [L, D], BF16, True)

    with tile.TileContext(nc) as tc:
        import contextlib
        ctx = contextlib.ExitStack()
        with ctx:
            res = ctx.enter_context(tc.tile_pool(name="res", bufs=1))
            wq_pool = ctx.enter_context(tc.tile_pool(name="wqk", bufs=8))
            rope_pool = ctx.enter_context(tc.tile_pool(name="rope", bufs=4))
            pt_pool = ctx.enter_context(tc.tile_pool(name="pt", bufs=10))
            y_pool = ctx.enter_context(tc.tile_pool(name="yt", bufs=4))

            xt_t = [res.tile([128, 4096], BF16, tag=f"xt{l4}", name=f"xt{l4}")
                    for l4 in range(4)]
            qkr = [res.tile([128, L], BF16, tag=f"qkr{c}", name=f"qkr{c}")
                   for c in range(8)]
            vsb = [res.tile([128, 520], BF16, tag=f"v{t}", name=f"v{t}")
                   for t in range(16)]
            wv_all = res.tile([128, 4096], BF16, tag="wv")
            wo_all = res.tile([128, 4096], BF16, tag="wo")
            cos_sb = res.tile([128, L], BF16, tag="cos")
            sin_sb = res.tile([128, L], BF16, tag="sin")
            tri_sb = res.tile([128, 128], BF16, tag="tri")
            nst = {n: res.tile([128, 512], FP32, tag=n, name=n)
                   for n in ("srowA", "srowB", "srow0A", "srow0B",
                             "avbl", "avs")}
            r0A = res.tile([128, 512], FP32, tag="r0A")
            r0B = res.tile([128, 512], FP32, tag="r0B")
            rbA = res.tile([128, 512], BF16, tag="rbA")
            rbB = res.tile([128, 512], BF16, tag="rbB")
            ones1 = res.tile([128, 64], BF16, tag="ones1")
            ot = [res.tile([128, L], BF16, tag=f"ot{p}", name=f"ot{p}")
                  for p in range(4)]

            def xta(d, sl):
                # global l-slice (within one 512 block) -> xt tile slice
                lt, q0 = divmod(sl.start, 512)
                n = sl.stop - sl.start
                assert q0 + n <= 512
                return xt_t[lt][:, 512 * d + q0:512 * d + q0 + n]

            # ones in every vsb slot; V copies overwrite cols 65h..65h+64
            for t in range(16):
                nc.vector.memset(vsb[t], 1.0)
            nc.vector.memset(ones1, 1.0)

            # ---- prologue DMAs, contiguous, ordered by first use ----
            wq = {}

            def load_w(c):
                w = wq_pool.tile([128, 1024], BF16, tag="w", name=f"w_{c}")
                nc.sync.dma_start(out=w, in_=wqk_d[c])
                wq[c] = w

            # The hardware runs up to 8 queued DMAs concurrently with
            # fair-shared HBM bandwidth, so the first compute-critical
            # transfers (w0, xt0 first half) would otherwise finish only
            # when the whole first batch does.  A tiny SBUF->SBUF dummy
            # DMA reading the critical tile stalls the queue until that
            # transfer completes, giving it the full bandwidth.
            prio = res.tile([1, 16], BF16, tag="prio")

            load_w(0)
            nc.sync.dma_start(out=xt_t[0][:, 0:2048], in_=xt_d[0][:, 0:2048])
            nc.sync.dma_start(out=prio, in_=xt_t[0][0:1, 0:16])
            nc.sync.dma_start(out=xt_t[0][:, 2048:4096],
                              in_=xt_d[0][:, 2048:4096])
            load_w(4)
            nc.sync.dma_start(out=cos_sb[:, 0:512], in_=cos_d[:, 0:512])
            nc.sync.dma_start(out=sin_sb[:, 0:512], in_=sin_d[:, 0:512])
            nc.sync.dma_start(out=prio, in_=xt_t[0][0:1, 2048:2064])
            nc.sync.dma_start(out=wv_all, in_=wv_d[:, :])
            nc.sync.dma_start(out=tri_sb, in_=tri_d[:, :])
            nc.sync.dma_start(out=prio, in_=wv_all[0:1, 0:16])
            load_w(1)
            load_w(5)
            nc.sync.dma_start(out=xt_t[1], in_=xt_d[1])
            load_w(2)
            load_w(6)
            load_w(3)
            load_w(7)
            nc.sync.dma_start(out=cos_sb[:, 512:2048], in_=cos_d[:, 512:2048])
            nc.sync.dma_start(out=sin_sb[:, 512:2048], in_=sin_d[:, 512:2048])
            nc.sync.dma_start(out=xt_t[2], in_=xt_d[2])
            nc.sync.dma_start(out=wo_all, in_=wo_d[:, :])
            nc.sync.dma_start(out=xt_t[3], in_=xt_d[3])

            # ---- emission helpers ----
            def emit_qk_l4(c, l4, qk_ps, tag="qkps"):
                w = wq[c]
                lsl = slice(512 * l4, 512 * l4 + 512)
                qkp = qk_ps.tile([128, 512], FP32, tag=tag)
                for d in range(8):
                    nc.tensor.matmul(qkp, w[:, 128 * d:128 * d + 128],
                                     xta(d, lsl), start=(d == 0),
                                     stop=(d == 7))
                shf = rope_pool.tile([128, 512], FP32, tag="shf")
                nc.vector.stream_shuffle(shf, qkp, SWAP_MASK)
                t1 = rope_pool.tile([128, 512], FP32, tag="t1")
                nc.vector.tensor_tensor(out=t1, in0=qkp, in1=cos_sb[:, lsl],
                                        op=MUL)
                t2 = rope_pool.tile([128, 512], FP32, tag="t2")
                # shf and sin are SBUF-resident, so this mul can run on
                # GpSimd, keeping the vector engine free for masks/norms
                nc.gpsimd.tensor_tensor(out=t2, in0=shf, in1=sin_sb[:, lsl],
                                        op=MUL)
                nc.gpsimd.tensor_tensor(out=qkr[c][:, lsl], in0=t1, in1=t2,
                                        op=ADD)

            def emit_v_tile(t, v_ps, tag="vps"):
                vp = v_ps.tile([128, 512], FP32, tag=tag)
                lsl = slice(128 * t, 128 * t + 128)
                for d in range(8):
                    nc.tensor.matmul(vp, xta(d, lsl),
                                     wv_all[:, 512 * d:512 * d + 512],
                                     start=(d == 0), stop=(d == 7))
                src = vp[:, :].rearrange("p (h e) -> p h e", h=8)
                dst = vsb[t][:, 0:520].rearrange("p (h e) -> p h e",
                                                 h=8, e=65)[:, :, 0:64]
                nc.scalar.copy(out=dst, in_=src)

            def emit_st_exp(jq, p, k, st_ps):
                qb0 = 512 * jq
                qt, kt = qkr[p], qkr[4 + p]
                kpos = 128 * k
                vs = max(0, kpos - qb0)
                st = st_ps.tile([128, 1024], FP32, tag="st")
                ksl = slice(kpos, kpos + 128)
                qsl = slice(qb0 + vs, qb0 + 512)
                nc.tensor.matmul(st[:, vs:512], kt[0:64, ksl], qt[0:64, qsl],
                                 start=True, stop=True, tile_position=(0, 0))
                nc.tensor.matmul(st[:, 512 + vs:1024], kt[64:128, ksl],
                                 qt[64:128, qsl], start=True, stop=True,
                                 tile_position=(64, 0))
                pt = pt_pool.tile([128, 1024], BF16, tag="pt")
                # strided AP: both heads' valid q ranges, skipping the
                # never-written [512:512+vs] junk between them
                src = st[:, :].rearrange("p (g q) -> p g q", g=2)[:, :, vs:512]
                dst = pt[:, :].rearrange("p (g q) -> p g q", g=2)[:, :, vs:512]
                nc.scalar.activation(out=dst, in_=src, func=EXP, scale=0.125)
                if kpos >= qb0:
                    dm = pt[:, :].rearrange("p (g q) -> p g q",
                                            g=2)[:, :, vs:vs + 128]
                    trib = tri_sb[:, :].unsqueeze(1).broadcast_to(
                        [128, 2, 128])
                    nc.vector.tensor_tensor(out=dm, in0=dm, in1=trib, op=MUL)
                return pt, vs

            def emit_av(p, k, nk, avA, avB, pt, vs):
                first, last = (k == 0), (k == nk - 1)
                isl = slice(vs, 512)
                ca = 130 * p
                nc.tensor.matmul(avA[0:65, isl], vsb[k][:, ca:ca + 65],
                                 pt[:, vs:512], start=first, stop=last)
                nc.tensor.matmul(avB[0:65, isl], vsb[k][:, ca + 65:ca + 130],
                                 pt[:, 512 + vs:1024], start=first, stop=last)

            def emit_norm(p, jq, avA, avB, rsb_ps, sceng=None,
                          on_group_done=None, avbl_eng=None):
                # PSUM is only DVE/ACT-readable; DVE cannot cross partition
                # offsets.  Stage s rows + av bulk off PSUM now (frees both av
                # banks), shift avB's partitions via SBUF->SBUF DMA, then in
                # the returned closure (fired a few ktiles into the NEXT
                # round) compute 1/s in place on partition 64 and BROADCAST it
                # across partitions with K=1 ones-matmuls at array row 64 into
                # a PSUM bank; per-head muls on DVE read that PSUM directly.
                qsl = slice(512 * jq, 512 * jq + 512)
                s = nst
                if sceng is None:
                    sceng = nc.scalar.copy
                sceng(out=s["srowA"][64:65, :], in_=avA[64:65, :])
                sceng(out=s["srowB"][64:65, :], in_=avB[64:65, :])
                nc.vector.tensor_copy(out=s["avs"][0:64, :],
                                      in_=avA[0:64, :])
                (avbl_eng or nc.vector.tensor_copy)(out=s["avbl"][0:64, :],
                                                    in_=avB[0:64, :])
                nc.sync.dma_start(out=s["srow0A"][0:1, :],
                                  in_=s["srowA"][64:65, :])
                nc.sync.dma_start(out=s["srow0B"][0:1, :],
                                  in_=s["srowB"][64:65, :])
                nc.sync.dma_start(out=s["avs"][64:128, :],
                                  in_=s["avbl"][0:64, :])

                def norm_head():
                    nc.vector.reciprocal_approx_fast(
                        out=r0A[0:1, :], in_=s["srow0A"][0:1, :])
                    nc.vector.reciprocal_approx_fast(
                        out=r0B[0:1, :], in_=s["srow0B"][0:1, :])
                    nc.vector.tensor_copy(out=rbA[0:1, :], in_=r0A[0:1, :])
                    nc.vector.tensor_copy(out=rbB[0:1, :], in_=r0B[0:1, :])
                    rp = rsb_ps.tile([128, 512], FP32, tag="rsb")
                    nc.tensor.matmul(rp[0:64, :], ones1[0:1, :],
                                     rbA[0:1, :], start=True, stop=True,
                                     tile_position=(0, 0))
                    nc.tensor.matmul(rp[64:128, :], ones1[0:1, :],
                                     rbB[0:1, :], start=True, stop=True,
                                     tile_position=(0, 64))

                    def norm_mul():
                        nc.vector.tensor_tensor(out=ot[p][:, qsl],
                                                in0=s["avs"][:, :],
                                                in1=rp[:, :], op=MUL)
                        # only once this write is emitted may the jq group's
                        # projection halves be queued (pops read ot[p])
                        if p == 3 and on_group_done is not None:
                            on_group_done(jq)
                    return norm_mul
                return norm_head

            def emit_proj_half(t, eh, y_ps, tag="yps", cast_eng=None):
                lsl = slice(128 * t, 128 * t + 128)
                yp = y_ps.tile([128, 512], FP32, tag=tag)
                for dc in range(4):
                    i = dc * 2 + eh
                    nc.tensor.matmul(yp, ot[dc][:, lsl],
                                     wo_all[:, 512 * i:512 * i + 512],
                                     start=(dc == 0), stop=(dc == 3))
                yt = y_pool.tile([128, 512], BF16, tag="yt")
                (cast_eng or nc.vector.tensor_copy)(out=yt, in_=yp)
                nc.sync.dma_start(out=y_d[lsl, 512 * eh:512 * eh + 512],
                                  in_=yt)

            pending_norm = [None]
            proj_queue = []

            def group_done(jq):
                proj_queue.extend(
                    (t, eh) for t in range(4 * jq, 4 * jq + 4)
                    for eh in range(2))

            def emit_round(jq, p, st_ps, av_ps, rsb_ps, skew, filler,
                           proj_cb=None, sceng=None, avbl_eng=None):
                """One attention round; pops one filler unit per ktile."""
                nk = 4 * (jq + 1)
                avA = av_ps.tile([128, 512], FP32, tag="avA",
                                 name=f"avA_{jq}_{p}")
                avB = av_ps.tile([128, 512], FP32, tag="avB",
                                 name=f"avB_{jq}_{p}")
                pending = []
                for k in range(nk):
                    pt, vs = emit_st_exp(jq, p, k, st_ps)
                    if k == 2 and pending_norm[0] is not None:
                        pending_norm[0] = pending_norm[0]()
                    elif k == 4 and pending_norm[0] is not None:
                        pending_norm[0]()
                        pending_norm[0] = None
                    if filler:
                        filler.pop(0)()
                    elif proj_cb is not None and k % 2 == 1:
                        proj_cb(1)
                    pending.append((k, pt, vs))
                    if len(pending) > skew:
                        kk, pp, vv = pending.pop(0)
                        emit_av(p, kk, nk, avA, avB, pp, vv)
                for kk, pp, vv in pending:
                    emit_av(p, kk, nk, avA, avB, pp, vv)
                if pending_norm[0] is not None:
                    pending_norm[0]()
                    pending_norm[0] = None
                pending_norm[0] = emit_norm(p, jq, avA, avB, rsb_ps,
                                            sceng, group_done, avbl_eng)

            # ---- phase 1: rounds jq=0,1 with qk/V units woven in ----
            rsb_ps = ctx.enter_context(
                tc.tile_pool(name="ps_rsb", bufs=1, space="PSUM"))
            with tc.tile_pool(name="ps_qk", bufs=2, space="PSUM") as qk_ps, \
                 tc.tile_pool(name="ps_v", bufs=1, space="PSUM") as v_ps, \
                 tc.tile_pool(name="ps_st1", bufs=1, space="PSUM") as st1_ps, \
                 tc.tile_pool(name="ps_av1", bufs=1, space="PSUM") as av1_ps:

                def qk_unit(c, l4):
                    return lambda: emit_qk_l4(c, l4, qk_ps)

                def v_unit(t):
                    return lambda: emit_v_tile(t, v_ps)

                def drain_proj1(n):
                    # group-0 proj halves reuse the V psum bank as filler;
                    # their output cast runs on the scalar engine, which has
                    # slack here, keeping DVE free for rope muls + masks
                    for _ in range(n):
                        if proj_queue:
                            emit_proj_half(*proj_queue.pop(0), v_ps,
                                           tag="vps",
                                           cast_eng=nc.scalar.copy)

                P1 = (lambda: drain_proj1(1))
                NULL = (lambda: None)

                # straight-line head: minimum needed to start round (0,0)
                emit_qk_l4(0, 0, qk_ps)
                emit_qk_l4(4, 0, qk_ps)
                for t in range(4):
                    emit_v_tile(t, v_ps)

                plan1 = [
                    (0, 0, [qk_unit(1, 0), qk_unit(5, 0), NULL, NULL]),
                    (0, 1, [qk_unit(2, 0), qk_unit(6, 0),
                            qk_unit(0, 1), qk_unit(4, 1)]),
                    (0, 2, [qk_unit(3, 0), qk_unit(7, 0),
                            qk_unit(1, 1), qk_unit(5, 1)]),
                    (0, 3, [qk_unit(2, 1), qk_unit(6, 1),
                            qk_unit(3, 1), qk_unit(7, 1)]),
                    (1, 0, [v_unit(4), v_unit(5), v_unit(6), v_unit(7),
                            qk_unit(0, 2), qk_unit(4, 2),
                            qk_unit(1, 2), qk_unit(5, 2)]),
                    (1, 1, [qk_unit(2, 2), qk_unit(6, 2),
                            qk_unit(3, 2), qk_unit(7, 2),
                            v_unit(8), v_unit(9), P1, P1]),
                    (1, 2, [v_unit(10), v_unit(11),
                            qk_unit(0, 3), qk_unit(4, 3),
                            P1, P1, P1, P1]),
                    (1, 3, [qk_unit(1, 3), qk_unit(5, 3),
                            P1, NULL, P1, NULL, NULL, NULL]),
                ]
                for jq, p, fillers in plan1:
                    # srow + avbl staging on the scalar engine: phase-1 DVE
                    # is loaded with rope muls and masks, ACT has slack
                    emit_round(jq, p, st1_ps, av1_ps, rsb_ps, 2, fillers,
                               avbl_eng=nc.scalar.copy)

            # ---- phase 2: jq=2,3 rounds (minus (2,3)) + projection ----
            # late-needed qk l4=3 chunks, V12-15 and the proj halves are the
            # PE filler here so the PE stays busy (and the HAM clock gate
            # stays released) through the exp-paced big rounds.
            with tc.tile_pool(name="ps_st2", bufs=2, space="PSUM") as st2_ps, \
                 tc.tile_pool(name="ps_av2", bufs=1, space="PSUM") as av2_ps, \
                 tc.tile_pool(name="ps_aux", bufs=1, space="PSUM") as aux_ps:

                def drain_proj(n):
                    for _ in range(n):
                        if proj_queue:
                            emit_proj_half(*proj_queue.pop(0), aux_ps,
                                           tag="aux")

                def qk_unit2(c, l4):
                    return lambda: emit_qk_l4(c, l4, aux_ps, tag="aux")

                def v_unit2(t):
                    return lambda: emit_v_tile(t, aux_ps, tag="aux")

                P = (lambda: drain_proj(1))
                plan2 = [
                    (2, 0, [qk_unit2(2, 3), qk_unit2(6, 3),
                            v_unit2(12), v_unit2(13), NULL, P, P, P,
                            NULL, NULL, NULL, NULL]),
                    (2, 1, [qk_unit2(3, 3), qk_unit2(7, 3),
                            v_unit2(14), v_unit2(15), P, NULL, NULL, P,
                            NULL, NULL, NULL, NULL]),
                    (2, 2, [P, NULL, NULL, NULL, P, NULL, NULL, NULL,
                            P, NULL, NULL, NULL]),
                    (2, 3, [NULL, NULL, NULL, NULL, NULL, NULL, NULL, NULL,
                            NULL, NULL, NULL, NULL]),
                    (3, 0, [NULL, NULL, NULL, NULL, NULL, P, NULL, NULL,
                            P, NULL, NULL, NULL, NULL, NULL, NULL, NULL]),
                    (3, 1, [P, NULL, NULL, NULL, NULL, NULL, NULL, NULL,
                            P, NULL, NULL, NULL, NULL, NULL, NULL, NULL]),
                    (3, 2, [P, NULL, NULL, NULL, NULL, NULL, NULL, NULL,
                            NULL, NULL, NULL, NULL, NULL, NULL, NULL, NULL]),
                ]
                for i2, (jq, p, fillers) in enumerate(plan2):
                    emit_round(jq, p, st2_ps, av2_ps, rsb_ps, 2, fillers,
                               sceng=nc.vector.tensor_copy)
                    if i2 < 3:
                        drain_proj(1)

            # ---- phase 3: last round (3,3) ----
            with tc.tile_pool(name="ps_st3", bufs=1, space="PSUM") as st3_ps, \
                 tc.tile_pool(name="ps_av3", bufs=1, space="PSUM") as av3_ps, \
                 tc.tile_pool(name="ps_y3", bufs=1, space="PSUM") as y3_ps:

                def drain_proj3(n):
                    for _ in range(n):
                        if proj_queue:
                            emit_proj_half(*proj_queue.pop(0), y3_ps)

                P3 = (lambda: drain_proj3(1))
                fillers3 = ([NULL] * 4 + [P3] + [NULL] * 11)
                emit_round(3, 3, st3_ps, av3_ps, rsb_ps, 2, fillers3,
                           sceng=nc.scalar.copy, avbl_eng=nc.scalar.copy)

            # ---- tail.  st/av/y3 pools are closed, freeing 7 PSUM banks.
            # Two leftover group-2 fulls plus six group-3 dc=0..2 partial
            # accumulations run while the final norm chain executes, keeping
            # the PE busy and the HAM clock released; after the norm each
            # partial needs only its single dc=3 matmul. ----
            with tc.tile_pool(name="ps_y4", bufs=7, space="PSUM") as y4_ps:
                lead = []
                while proj_queue and len(lead) < 2:
                    lead.append(proj_queue.pop(0))
                for t, eh in lead:
                    emit_proj_half(t, eh, y4_ps, tag="y4")
                PART = [(12, 0), (12, 1), (13, 0), (13, 1), (14, 0), (14, 1)]
                parts = []
                for t, eh in PART:
                    yp = y4_ps.tile([128, 512], FP32, tag="y4",
                                    name=f"ypre{t}_{eh}")
                    for dc in range(3):
                        i = dc * 2 + eh
                        nc.tensor.matmul(yp, ot[dc][:, 128 * t:128 * t + 128],
                                         wo_all[:, 512 * i:512 * i + 512],
                                         start=(dc == 0), stop=False)
                    parts.append((t, eh, yp))
                nh_mul = pending_norm[0]()   # recips/casts + rsb matmuls
                pending_norm[0] = None
                nh_mul()                     # final ot write; queues group 3
                for n4, (t, eh, yp) in enumerate(parts):
                    i = 3 * 2 + eh
                    nc.tensor.matmul(yp, ot[3][:, 128 * t:128 * t + 128],
                                     wo_all[:, 512 * i:512 * i + 512],
                                     start=False, stop=True)
                    yt = y_pool.tile([128, 512], BF16, tag="yt")
                    # alternate cast engines so the output drain pipelines
                    (nc.scalar.copy if n4 % 2 else
                     nc.vector.tensor_copy)(out=yt, in_=yp)
                    nc.sync.dma_start(
                        out=y_d[128 * t:128 * t + 128,
                                512 * eh:512 * eh + 512], in_=yt)
                for n4, (t, eh) in enumerate(proj_queue):
                    if (t, eh) not in PART:
                        emit_proj_half(t, eh, y4_ps, tag="y4",
                                       cast_eng=(nc.scalar.copy if n4 % 2
                                                 else None))
    nc.compile()
    return nc


def _get_nc():
    global _built
    if _built is None:
        _built = _build()
    return _built


def _in_maps(x, W, Wo):
    x = np.asarray(x, np.float32)
    W = np.asarray(W, np.float32)
    Wo = np.asarray(Wo, np.float32)

    cos2, sin2 = _rope_tables()
    tri = np.zeros((128, 128), np.float32)
    p_idx = np.arange(128)
    tri[p_idx[:, None] <= p_idx[None, :]] = 1.0  # valid: k <= q
    tri = tri.astype(BF)

    in_maps = []
    for core in range(NCORES):
        b, g = core // 2, core % 2
        xt = np.ascontiguousarray(x[b].T).astype(BF)                 # [D, L]
        # -> [l4, 128(d-part), 8(dchunk) x 512] contiguous
        xt = xt.reshape(8, 128, 4, 512).transpose(2, 1, 0, 3)
        xt = np.ascontiguousarray(xt).reshape(4, 128, 4096)
        wq = W[512 * g:512 * g + 512]                                # [512, D]
        wk = W[D + 512 * g:D + 512 * g + 512]
        wv = W[2 * D + 512 * g:2 * D + 512 * g + 512]
        wqk_t = np.ascontiguousarray(
            np.concatenate([wq, wk], 0).T).astype(BF)                # [D, 1024]
        # -> [echunk, 128(d-part), 1024(8 dchunk x 128 e)]
        wqk_t = wqk_t.reshape(8, 128, 8, 128).transpose(2, 1, 0, 3)
        wqk_t = np.ascontiguousarray(wqk_t.reshape(8, 128, 1024))
        wv_t = np.ascontiguousarray(wv.T).astype(BF)                 # [D, 512]
        wv_t = np.ascontiguousarray(
            wv_t.reshape(8, 128, 512).transpose(1, 0, 2))            # [128,8,512]
        wv_t = wv_t.reshape(128, 4096)
        wo_t = np.ascontiguousarray(Wo[:, 512 * g:512 * g + 512].T).astype(BF)
        wo_t = wo_t.reshape(4, 128, 2, 512).transpose(1, 0, 2, 3)    # [128,4,2,512]
        wo_t = np.ascontiguousarray(wo_t).reshape(128, 4096)
        in_maps.append({
            "xt": xt, "wqk": wqk_t, "wv": wv_t, "wo": wo_t,
            "cos2": cos2, "sin2": sin2, "trimask": tri,
        })
    return in_maps


def kernel(x, W, Wo):
    from concourse.bass_utils import run_bass_kernel_spmd

    res = run_bass_kernel_spmd(_get_nc(), _in_maps(x, W, Wo),
                               list(range(NCORES)))
    out = np.empty((B, L, D), np.float32)
    for b in range(B):
        out[b] = (res.results[2 * b]["y"].astype(np.float32)
                  + res.results[2 * b + 1]["y"].astype(np.float32))
    return out


def _install_ntff_hook_shim():
    """The trimmed repo lacks antenv.axon_hooks; reconstruct it so
    run_bass_kernel_spmd(trace=True) can NTFF-profile through axon."""
    import sys as _sys, types
    if "antenv.axon_hooks" in _sys.modules:
        return
    import antenv  # noqa: F401
    from trn_agent_boot.trn_boot import _ntff_profile_via_ctypes
    hook = _ntff_profile_via_ctypes("/opt/axon/libaxon_pjrt.so")
    mod = types.ModuleType("antenv.axon_hooks")
    mod.set_axon_ntff_profile_hook = lambda h: None
    mod.get_axon_ntff_profile_hook = lambda: hook
    _sys.modules["antenv.axon_hooks"] = mod


def kernel_traced(x, W, Wo, tmpdir=None):
    """Run with NTFF tracing; returns exec time ns (trace in tmpdir)."""
    from concourse.bass_utils import run_bass_kernel_spmd

    _install_ntff_hook_shim()
    res = run_bass_kernel_spmd(_get_nc(), _in_maps(x, W, Wo),
                               list(range(NCORES)), trace=True, tmpdir=tmpdir)
    return res.exec_time_ns


# revision 33
# speedup vs baseline: 1.0094x; 1.0094x over previous
"""Multi-head self-attention (B=4, L=2048, D=1024, H=16, RoPE, causal) on 8
Trainium2 NeuronCores.

Sharding: data-parallel over batch (4) x tensor-parallel over head groups (2).
Core i handles batch i//2, heads 8*(i%2) .. 8*(i%2)+8.  Each core computes its
QKV projection slice, RoPE, causal attention for its 8 heads, and a partial
output projection over its 512 d-columns; the host sums the two partials per
batch.

On-core dataflow (per core, matmul operands bf16, psum/softmax fp32):
  qkT[e,l] = Wqk_sub @ x^T       (e = 8 q-heads then 8 k-heads, dh-major)
  rope on qkT rows (pair-swap via stream_shuffle + bf16 cos/sin tables)
  V[l,e]   = x @ Wv_sub^T        stored with a ones column per head:
             vsb[k][:, 65h:65h+64] = V_h, vsb[k][:, 65h+64] = 1
  per head-pair p (heads 2p, 2p+1), per 512-wide q block:
    S^T[k,q] = K^T Q  (row-split pair, tile_position (0,0)/(64,0))
    P^T = exp(S^T/8), one contiguous ACT op + tri-mask per head
    av[0:64] / s[64] = [V_h | ones]^T @ P^T_h   (full-array, 65-col stationary
                       -> AV and the softmax denominator in one stream pass)
    norm: s rows staged to SBUF partition 64, 1/s in place on partition 64
    (DVE), broadcast across partitions via K=1 ones-matmuls at array row 64
    into PSUM, per-head muls on DVE
  y[l,e] partial = O^T.T @ Wo_sub  (lhsT = O^T chunks), bf16 output

Scheduling: demand-driven.  The straight-line head is only qk chunks c0/c4
l4=0 plus V0-3; every other qk chunk / V tile is woven into the attention
rounds as PE filler so the scalar engine (exp, the attention-phase pacer)
starts ~15us in and the PE never idles long enough for the HAM clock gate to
re-throttle mid-kernel.  Each round's normalization tail is deferred into the
next round.  Inputs are host-packed contiguous so every prologue DMA is one
descriptor per partition, ordered by first use.
"""
import sys
sys.path.insert(0, "/opt/trn_rl_repo")

import numpy as np
import ml_dtypes

B, L, D, H = 4, 2048, 1024, 16
DH = D // H  # 64
THETA = 100000.0
NCORES = 8
BF = ml_dtypes.bfloat16

_built = None


def _rope_tables():
    # [128, L] bf16: rows = 2 stacked heads' dh (64 each), identical per head.
    pos = np.arange(L, dtype=np.float32)
    inv_freq = (1.0 / THETA ** (np.arange(0, DH, 2, dtype=np.float32) / DH))
    ang = pos[None, :] * inv_freq[:, None]              # [32, L]
    cos = np.cos(ang)                                    # [32, L]
    sin = np.sin(ang)
    cos2 = np.repeat(cos, 2, axis=0)                     # rows 2p,2p+1 = cos_p
    sin2 = np.empty((DH, L), np.float32)
    sin2[0::2] = -sin
    sin2[1::2] = sin
    return (np.concatenate([cos2, cos2], 0).astype(BF),
            np.concatenate([sin2, sin2], 0).astype(BF))


def _build():
    import concourse.mybir as mybir
    import concourse.tile as tile
    from concourse import bacc

    FP32 = mybir.dt.float32
    BF16 = mybir.dt.bfloat16
    MUL = mybir.AluOpType.mult
    ADD = mybir.AluOpType.add
    EXP = mybir.ActivationFunctionType.Exp
    SWAP_MASK = [i ^ 1 for i in range(32)]

    nc = bacc.Bacc(None, target_bir_lowering=False)
    # host-packed layouts, fully contiguous per DMA
    xt_d = nc.declare_dram_parameter("xt", [4, 128, 4096], BF16, False)
    wqk_d = nc.declare_dram_parameter("wqk", [8, 128, 1024], BF16, False)
    wv_d = nc.declare_dram_parameter("wv", [128, 4096], BF16, False)
    wo_d = nc.declare_dram_parameter("wo", [128, 4096], BF16, False)
    cos_d = nc.declare_dram_parameter("cos2", [128, L], BF16, False)
    sin_d = nc.declare_dram_parameter("sin2", [128, L], BF16, False)
    tri_d = nc.declare_dram_parameter("trimask", [128, 128], BF16, False)
    # y stored as contiguous [t, eh, 128, 512] blocks (host reassembles):
    # strided [L, D] writes halved the output-drain DMA bandwidth
    y_d = nc.declare_dram_parameter("y", [16, 2, 128, 512], BF16, True)

    with tile.TileContext(nc) as tc:
        import contextlib
        ctx = contextlib.ExitStack()
        with ctx:
            res = ctx.enter_context(tc.tile_pool(name="res", bufs=1))
            wq_pool = ctx.enter_context(tc.tile_pool(name="wqk", bufs=8))
            rope_pool = ctx.enter_context(tc.tile_pool(name="rope", bufs=4))
            pt_pool = ctx.enter_context(tc.tile_pool(name="pt", bufs=10))
            y_pool = ctx.enter_context(tc.tile_pool(name="yt", bufs=4))

            xt_t = [res.tile([128, 4096], BF16, tag=f"xt{l4}", name=f"xt{l4}")
                    for l4 in range(4)]
            qkr = [res.tile([128, L], BF16, tag=f"qkr{c}", name=f"qkr{c}")
                   for c in range(8)]
            vsb = [res.tile([128, 520], BF16, tag=f"v{t}", name=f"v{t}")
                   for t in range(16)]
            wv_all = res.tile([128, 4096], BF16, tag="wv")
            wo_all = res.tile([128, 4096], BF16, tag="wo")
            cos_sb = res.tile([128, L], BF16, tag="cos")
            sin_sb = res.tile([128, L], BF16, tag="sin")
            tri_sb = res.tile([128, 128], BF16, tag="tri")
            nst = {n: res.tile([128, 512], FP32, tag=n, name=n)
                   for n in ("srowA", "srowB", "srow0A", "srow0B",
                             "avbl", "avs")}
            r0A = res.tile([128, 512], FP32, tag="r0A")
            r0B = res.tile([128, 512], FP32, tag="r0B")
            rbA = res.tile([128, 512], BF16, tag="rbA")
            rbB = res.tile([128, 512], BF16, tag="rbB")
            ones1 = res.tile([128, 64], BF16, tag="ones1")
            ot = [res.tile([128, L], BF16, tag=f"ot{p}", name=f"ot{p}")
                  for p in range(4)]

            def xta(d, sl):
                # global l-slice (within one 512 block) -> xt tile slice
                lt, q0 = divmod(sl.start, 512)
                n = sl.stop - sl.start
                assert q0 + n <= 512
                return xt_t[lt][:, 512 * d + q0:512 * d + q0 + n]

            # ones in every vsb slot; V copies overwrite cols 65h..65h+64
            for t in range(16):
                nc.vector.memset(vsb[t], 1.0)
            nc.vector.memset(ones1, 1.0)

            # ---- prologue DMAs, contiguous, ordered by first use ----
            wq = {}

            def load_w(c):
                w = wq_pool.tile([128, 1024], BF16, tag="w", name=f"w_{c}")
                nc.sync.dma_start(out=w, in_=wqk_d[c])
                wq[c] = w

            # The hardware runs up to 8 queued DMAs concurrently with
            # fair-shared HBM bandwidth, so the first compute-critical
            # transfers (w0, xt0 first half) would otherwise finish only
            # when the whole first batch does.  A tiny SBUF->SBUF dummy
            # DMA reading the critical tile stalls the queue until that
            # transfer completes, giving it the full bandwidth.
            prio = res.tile([1, 16], BF16, tag="prio")

            load_w(0)
            nc.sync.dma_start(out=xt_t[0][:, 0:2048], in_=xt_d[0][:, 0:2048])
            nc.sync.dma_start(out=prio, in_=xt_t[0][0:1, 0:16])
            nc.sync.dma_start(out=xt_t[0][:, 2048:4096],
                              in_=xt_d[0][:, 2048:4096])
            load_w(4)
            nc.sync.dma_start(out=cos_sb[:, 0:512], in_=cos_d[:, 0:512])
            nc.sync.dma_start(out=sin_sb[:, 0:512], in_=sin_d[:, 0:512])
            nc.sync.dma_start(out=prio, in_=xt_t[0][0:1, 2048:2064])
            nc.sync.dma_start(out=wv_all, in_=wv_d[:, :])
            nc.sync.dma_start(out=tri_sb, in_=tri_d[:, :])
            nc.sync.dma_start(out=prio, in_=wv_all[0:1, 0:16])
            load_w(1)
            load_w(5)
            nc.sync.dma_start(out=xt_t[1], in_=xt_d[1])
            load_w(2)
            load_w(6)
            load_w(3)
            load_w(7)
            nc.sync.dma_start(out=cos_sb[:, 512:2048], in_=cos_d[:, 512:2048])
            nc.sync.dma_start(out=sin_sb[:, 512:2048], in_=sin_d[:, 512:2048])
            nc.sync.dma_start(out=xt_t[2], in_=xt_d[2])
            nc.sync.dma_start(out=wo_all, in_=wo_d[:, :])
            nc.sync.dma_start(out=xt_t[3], in_=xt_d[3])

            # ---- emission helpers ----
            def emit_qk_l4(c, l4, qk_ps, tag="qkps"):
                w = wq[c]
                lsl = slice(512 * l4, 512 * l4 + 512)
                qkp = qk_ps.tile([128, 512], FP32, tag=tag)
                for d in range(8):
                    nc.tensor.matmul(qkp, w[:, 128 * d:128 * d + 128],
                                     xta(d, lsl), start=(d == 0),
                                     stop=(d == 7))
                shf = rope_pool.tile([128, 512], FP32, tag="shf")
                nc.vector.stream_shuffle(shf, qkp, SWAP_MASK)
                t1 = rope_pool.tile([128, 512], FP32, tag="t1")
                nc.vector.tensor_tensor(out=t1, in0=qkp, in1=cos_sb[:, lsl],
                                        op=MUL)
                t2 = rope_pool.tile([128, 512], FP32, tag="t2")
                # shf and sin are SBUF-resident, so this mul can run on
                # GpSimd, keeping the vector engine free for masks/norms
                nc.gpsimd.tensor_tensor(out=t2, in0=shf, in1=sin_sb[:, lsl],
                                        op=MUL)
                nc.gpsimd.tensor_tensor(out=qkr[c][:, lsl], in0=t1, in1=t2,
                                        op=ADD)

            def emit_v_tile(t, v_ps, tag="vps"):
                vp = v_ps.tile([128, 512], FP32, tag=tag)
                lsl = slice(128 * t, 128 * t + 128)
                for d in range(8):
                    nc.tensor.matmul(vp, xta(d, lsl),
                                     wv_all[:, 512 * d:512 * d + 512],
                                     start=(d == 0), stop=(d == 7))
                src = vp[:, :].rearrange("p (h e) -> p h e", h=8)
                dst = vsb[t][:, 0:520].rearrange("p (h e) -> p h e",
                                                 h=8, e=65)[:, :, 0:64]
                nc.scalar.copy(out=dst, in_=src)

            def emit_st_exp(jq, p, k, st_ps):
                qb0 = 512 * jq
                qt, kt = qkr[p], qkr[4 + p]
                kpos = 128 * k
                vs = max(0, kpos - qb0)
                st = st_ps.tile([128, 1024], FP32, tag="st")
                ksl = slice(kpos, kpos + 128)
                qsl = slice(qb0 + vs, qb0 + 512)
                nc.tensor.matmul(st[:, vs:512], kt[0:64, ksl], qt[0:64, qsl],
                                 start=True, stop=True, tile_position=(0, 0))
                nc.tensor.matmul(st[:, 512 + vs:1024], kt[64:128, ksl],
                                 qt[64:128, qsl], start=True, stop=True,
                                 tile_position=(64, 0))
                pt = pt_pool.tile([128, 1024], BF16, tag="pt")
                # strided AP: both heads' valid q ranges, skipping the
                # never-written [512:512+vs] junk between them
                src = st[:, :].rearrange("p (g q) -> p g q", g=2)[:, :, vs:512]
                dst = pt[:, :].rearrange("p (g q) -> p g q", g=2)[:, :, vs:512]
                nc.scalar.activation(out=dst, in_=src, func=EXP, scale=0.125)
                if kpos >= qb0:
                    dm = pt[:, :].rearrange("p (g q) -> p g q",
                                            g=2)[:, :, vs:vs + 128]
                    trib = tri_sb[:, :].unsqueeze(1).broadcast_to(
                        [128, 2, 128])
                    nc.vector.tensor_tensor(out=dm, in0=dm, in1=trib, op=MUL)
                return pt, vs

            def emit_av(p, k, nk, avA, avB, pt, vs):
                first, last = (k == 0), (k == nk - 1)
                isl = slice(vs, 512)
                ca = 130 * p
                nc.tensor.matmul(avA[0:65, isl], vsb[k][:, ca:ca + 65],
                                 pt[:, vs:512], start=first, stop=last)
                nc.tensor.matmul(avB[0:65, isl], vsb[k][:, ca + 65:ca + 130],
                                 pt[:, 512 + vs:1024], start=first, stop=last)

            def emit_norm(p, jq, avA, avB, rsb_ps, sceng=None,
                          on_group_done=None, avbl_eng=None):
                # PSUM is only DVE/ACT-readable; DVE cannot cross partition
                # offsets.  Stage s rows + av bulk off PSUM now (frees both av
                # banks), shift avB's partitions via SBUF->SBUF DMA, then in
                # the returned closure (fired a few ktiles into the NEXT
                # round) compute 1/s in place on partition 64 and BROADCAST it
                # across partitions with K=1 ones-matmuls at array row 64 into
                # a PSUM bank; per-head muls on DVE read that PSUM directly.
                qsl = slice(512 * jq, 512 * jq + 512)
                s = nst
                if sceng is None:
                    sceng = nc.scalar.copy
                sceng(out=s["srowA"][64:65, :], in_=avA[64:65, :])
                sceng(out=s["srowB"][64:65, :], in_=avB[64:65, :])
                nc.vector.tensor_copy(out=s["avs"][0:64, :],
                                      in_=avA[0:64, :])
                (avbl_eng or nc.vector.tensor_copy)(out=s["avbl"][0:64, :],
                                                    in_=avB[0:64, :])
                nc.sync.dma_start(out=s["srow0A"][0:1, :],
                                  in_=s["srowA"][64:65, :])
                nc.sync.dma_start(out=s["srow0B"][0:1, :],
                                  in_=s["srowB"][64:65, :])
                nc.sync.dma_start(out=s["avs"][64:128, :],
                                  in_=s["avbl"][0:64, :])

                def norm_head():
                    nc.vector.reciprocal_approx_fast(
                        out=r0A[0:1, :], in_=s["srow0A"][0:1, :])
                    nc.vector.reciprocal_approx_fast(
                        out=r0B[0:1, :], in_=s["srow0B"][0:1, :])
                    nc.vector.tensor_copy(out=rbA[0:1, :], in_=r0A[0:1, :])
                    nc.vector.tensor_copy(out=rbB[0:1, :], in_=r0B[0:1, :])
                    rp = rsb_ps.tile([128, 512], FP32, tag="rsb")
                    nc.tensor.matmul(rp[0:64, :], ones1[0:1, :],
                                     rbA[0:1, :], start=True, stop=True,
                                     tile_position=(0, 0))
                    nc.tensor.matmul(rp[64:128, :], ones1[0:1, :],
                                     rbB[0:1, :], start=True, stop=True,
                                     tile_position=(0, 64))

                    def norm_mul():
                        nc.vector.tensor_tensor(out=ot[p][:, qsl],
                                                in0=s["avs"][:, :],
                                                in1=rp[:, :], op=MUL)
                        # only once this write is emitted may the jq group's
                        # projection halves be queued (pops read ot[p])
                        if p == 3 and on_group_done is not None:
                            on_group_done(jq)
                    return norm_mul
                return norm_head

            def emit_proj_half(t, eh, y_ps, tag="yps", cast_eng=None):
                lsl = slice(128 * t, 128 * t + 128)
                yp = y_ps.tile([128, 512], FP32, tag=tag)
                for dc in range(4):
                    i = dc * 2 + eh
                    nc.tensor.matmul(yp, ot[dc][:, lsl],
                                     wo_all[:, 512 * i:512 * i + 512],
                                     start=(dc == 0), stop=(dc == 3))
                yt = y_pool.tile([128, 512], BF16, tag="yt")
                (cast_eng or nc.vector.tensor_copy)(out=yt, in_=yp)
                nc.sync.dma_start(out=y_d[t, eh], in_=yt)

            pending_norm = [None]
            proj_queue = []

            def group_done(jq):
                proj_queue.extend(
                    (t, eh) for t in range(4 * jq, 4 * jq + 4)
                    for eh in range(2))

            def emit_round(jq, p, st_ps, av_ps, rsb_ps, skew, filler,
                           proj_cb=None, sceng=None, avbl_eng=None):
                """One attention round; pops one filler unit per ktile."""
                nk = 4 * (jq + 1)
                avA = av_ps.tile([128, 512], FP32, tag="avA",
                                 name=f"avA_{jq}_{p}")
                avB = av_ps.tile([128, 512], FP32, tag="avB",
                                 name=f"avB_{jq}_{p}")
                pending = []
                for k in range(nk):
                    pt, vs = emit_st_exp(jq, p, k, st_ps)
                    if k == 2 and pending_norm[0] is not None:
                        pending_norm[0] = pending_norm[0]()
                    elif k == 4 and pending_norm[0] is not None:
                        pending_norm[0]()
                        pending_norm[0] = None
                    if filler:
                        filler.pop(0)()
                    elif proj_cb is not None and k % 2 == 1:
                        proj_cb(1)
                    pending.append((k, pt, vs))
                    if len(pending) > skew:
                        kk, pp, vv = pending.pop(0)
                        emit_av(p, kk, nk, avA, avB, pp, vv)
                for kk, pp, vv in pending:
                    emit_av(p, kk, nk, avA, avB, pp, vv)
                if pending_norm[0] is not None:
                    pending_norm[0]()
                    pending_norm[0] = None
                pending_norm[0] = emit_norm(p, jq, avA, avB, rsb_ps,
                                            sceng, group_done, avbl_eng)

            # ---- phase 1: rounds jq=0,1 with qk/V units woven in ----
            rsb_ps = ctx.enter_context(
                tc.tile_pool(name="ps_rsb", bufs=1, space="PSUM"))
            with tc.tile_pool(name="ps_qk", bufs=2, space="PSUM") as qk_ps, \
                 tc.tile_pool(name="ps_v", bufs=1, space="PSUM") as v_ps, \
                 tc.tile_pool(name="ps_st1", bufs=1, space="PSUM") as st1_ps, \
                 tc.tile_pool(name="ps_av1", bufs=1, space="PSUM") as av1_ps:

                def qk_unit(c, l4):
                    return lambda: emit_qk_l4(c, l4, qk_ps)

                def v_unit(t):
                    return lambda: emit_v_tile(t, v_ps)

                def drain_proj1(n):
                    # group-0 proj halves reuse the V psum bank as filler;
                    # their output cast runs on the scalar engine, which has
                    # slack here, keeping DVE free for rope muls + masks
                    for _ in range(n):
                        if proj_queue:
                            emit_proj_half(*proj_queue.pop(0), v_ps,
                                           tag="vps",
                                           cast_eng=nc.scalar.copy)

                P1 = (lambda: drain_proj1(1))
                NULL = (lambda: None)

                # straight-line head: minimum needed to start round (0,0)
                emit_qk_l4(0, 0, qk_ps)
                emit_qk_l4(4, 0, qk_ps)
                for t in range(4):
                    emit_v_tile(t, v_ps)

                plan1 = [
                    (0, 0, [qk_unit(1, 0), qk_unit(5, 0), NULL, NULL]),
                    (0, 1, [qk_unit(2, 0), qk_unit(6, 0),
                            qk_unit(0, 1), qk_unit(4, 1)]),
                    (0, 2, [qk_unit(3, 0), qk_unit(7, 0),
                            qk_unit(1, 1), qk_unit(5, 1)]),
                    (0, 3, [qk_unit(2, 1), qk_unit(6, 1),
                            qk_unit(3, 1), qk_unit(7, 1)]),
                    (1, 0, [v_unit(4), v_unit(5), v_unit(6), v_unit(7),
                            qk_unit(0, 2), qk_unit(4, 2),
                            qk_unit(1, 2), qk_unit(5, 2)]),
                    (1, 1, [qk_unit(2, 2), qk_unit(6, 2),
                            qk_unit(3, 2), qk_unit(7, 2),
                            v_unit(8), v_unit(9),
                            qk_unit(0, 3), qk_unit(4, 3)]),
                ]
                for jq, p, fillers in plan1:
                    # srow + avbl staging on the scalar engine: phase-1 DVE
                    # is loaded with rope muls and masks, ACT has slack
                    emit_round(jq, p, st1_ps, av1_ps, rsb_ps, 2, fillers,
                               avbl_eng=nc.scalar.copy)

            # ---- phase 2: jq=2,3 rounds (minus (2,3)) + projection ----
            # late-needed qk l4=3 chunks, V12-15 and the proj halves are the
            # PE filler here so the PE stays busy (and the HAM clock gate
            # stays released) through the exp-paced big rounds.
            with tc.tile_pool(name="ps_st2", bufs=2, space="PSUM") as st2_ps, \
                 tc.tile_pool(name="ps_av2", bufs=1, space="PSUM") as av2_ps, \
                 tc.tile_pool(name="ps_aux", bufs=1, space="PSUM") as aux_ps:

                def drain_proj(n):
                    for _ in range(n):
                        if proj_queue:
                            emit_proj_half(*proj_queue.pop(0), aux_ps,
                                           tag="aux")

                def qk_unit2(c, l4):
                    return lambda: emit_qk_l4(c, l4, aux_ps, tag="aux")

                def v_unit2(t):
                    return lambda: emit_v_tile(t, aux_ps, tag="aux")

                P = (lambda: drain_proj(1))
                plan2 = [
                    (1, 2, [v_unit2(10), v_unit2(11),
                            qk_unit2(1, 3), qk_unit2(5, 3),
                            P, P, P, P]),
                    (1, 3, [qk_unit2(2, 3), qk_unit2(6, 3), P, P,
                            v_unit2(12), v_unit2(13), P, P]),
                    (2, 0, [qk_unit2(3, 3), qk_unit2(7, 3),
                            v_unit2(14), v_unit2(15), NULL, P, P, NULL,
                            P, NULL, NULL, NULL]),
                    (2, 1, [P, NULL, NULL, NULL, P, NULL, NULL, NULL,
                            P, NULL, NULL, NULL]),
                    (2, 2, [P, NULL, NULL, NULL, P, NULL, NULL, NULL,
                            NULL, NULL, NULL, NULL]),
                    (2, 3, [NULL, NULL, NULL, NULL, NULL, NULL, NULL, NULL,
                            NULL, NULL, NULL, NULL]),
                    (3, 0, [NULL, NULL, NULL, NULL, NULL, P, NULL, NULL,
                            P, NULL, NULL, NULL, NULL, NULL, NULL, NULL]),
                    (3, 1, [P, NULL, NULL, NULL, NULL, NULL, NULL, NULL,
                            P, NULL, NULL, NULL, NULL, NULL, NULL, NULL]),
                    (3, 2, [P, NULL, NULL, NULL, NULL, NULL, NULL, NULL,
                            NULL, NULL, NULL, NULL, NULL, NULL, NULL, NULL]),
                ]
                for jq, p, fillers in plan2:
                    emit_round(jq, p, st2_ps, av2_ps, rsb_ps, 2, fillers,
                               sceng=nc.vector.tensor_copy)

            # ---- phase 3: last round (3,3) ----
            with tc.tile_pool(name="ps_st3", bufs=1, space="PSUM") as st3_ps, \
                 tc.tile_pool(name="ps_av3", bufs=1, space="PSUM") as av3_ps, \
                 tc.tile_pool(name="ps_y3", bufs=1, space="PSUM") as y3_ps:

                def drain_proj3(n):
                    for _ in range(n):
                        if proj_queue:
                            emit_proj_half(*proj_queue.pop(0), y3_ps)

                P3 = (lambda: drain_proj3(1))
                fillers3 = ([NULL] * 4 + [P3] + [NULL] * 11)
                emit_round(3, 3, st3_ps, av3_ps, rsb_ps, 2, fillers3,
                           sceng=nc.scalar.copy, avbl_eng=nc.scalar.copy)

            # ---- tail.  st/av/y3 pools are closed, freeing 7 PSUM banks.
            # Two leftover group-2 fulls plus six group-3 dc=0..2 partial
            # accumulations run while the final norm chain executes, keeping
            # the PE busy and the HAM clock released; after the norm each
            # partial needs only its single dc=3 matmul. ----
            with tc.tile_pool(name="ps_y4", bufs=7, space="PSUM") as y4_ps:
                lead = []
                while proj_queue and len(lead) < 2:
                    lead.append(proj_queue.pop(0))
                for t, eh in lead:
                    emit_proj_half(t, eh, y4_ps, tag="y4")
                PART = [(12, 0), (12, 1), (13, 0), (13, 1), (14, 0), (14, 1)]
                parts = []
                for t, eh in PART:
                    yp = y4_ps.tile([128, 512], FP32, tag="y4",
                                    name=f"ypre{t}_{eh}")
                    for dc in range(3):
                        i = dc * 2 + eh
                        nc.tensor.matmul(yp, ot[dc][:, 128 * t:128 * t + 128],
                                         wo_all[:, 512 * i:512 * i + 512],
                                         start=(dc == 0), stop=False)
                    parts.append((t, eh, yp))
                nh_mul = pending_norm[0]()   # recips/casts + rsb matmuls
                pending_norm[0] = None
                nh_mul()                     # final ot write; queues group 3
                for n4, (t, eh, yp) in enumerate(parts):
                    i = 3 * 2 + eh
                    nc.tensor.matmul(yp, ot[3][:, 128 * t:128 * t + 128],
                                     wo_all[:, 512 * i:512 * i + 512],
                                     start=False, stop=True)
                    yt = y_pool.tile([128, 512], BF16, tag="yt")
                    # alternate cast engines so the output drain pipelines
                    (nc.scalar.copy if n4 % 2 else
                     nc.vector.tensor_copy)(out=yt, in_=yp)
                    nc.sync.dma_start(out=y_d[t, eh], in_=yt)
                for n4, (t, eh) in enumerate(proj_queue):
                    if (t, eh) not in PART:
                        emit_proj_half(t, eh, y4_ps, tag="y4",
                                       cast_eng=(nc.scalar.copy if n4 % 2
                                                 else None))
    nc.compile()
    return nc


def _get_nc():
    global _built
    if _built is None:
        _built = _build()
    return _built


def _in_maps(x, W, Wo):
    x = np.asarray(x, np.float32)
    W = np.asarray(W, np.float32)
    Wo = np.asarray(Wo, np.float32)

    cos2, sin2 = _rope_tables()
    tri = np.zeros((128, 128), np.float32)
    p_idx = np.arange(128)
    tri[p_idx[:, None] <= p_idx[None, :]] = 1.0  # valid: k <= q
    tri = tri.astype(BF)

    in_maps = []
    for core in range(NCORES):
        b, g = core // 2, core % 2
        xt = np.ascontiguousarray(x[b].T).astype(BF)                 # [D, L]
        # -> [l4, 128(d-part), 8(dchunk) x 512] contiguous
        xt = xt.reshape(8, 128, 4, 512).transpose(2, 1, 0, 3)
        xt = np.ascontiguousarray(xt).reshape(4, 128, 4096)
        wq = W[512 * g:512 * g + 512]                                # [512, D]
        wk = W[D + 512 * g:D + 512 * g + 512]
        wv = W[2 * D + 512 * g:2 * D + 512 * g + 512]
        wqk_t = np.ascontiguousarray(
            np.concatenate([wq, wk], 0).T).astype(BF)                # [D, 1024]
        # -> [echunk, 128(d-part), 1024(8 dchunk x 128 e)]
        wqk_t = wqk_t.reshape(8, 128, 8, 128).transpose(2, 1, 0, 3)
        wqk_t = np.ascontiguousarray(wqk_t.reshape(8, 128, 1024))
        wv_t = np.ascontiguousarray(wv.T).astype(BF)                 # [D, 512]
        wv_t = np.ascontiguousarray(
            wv_t.reshape(8, 128, 512).transpose(1, 0, 2))            # [128,8,512]
        wv_t = wv_t.reshape(128, 4096)
        wo_t = np.ascontiguousarray(Wo[:, 512 * g:512 * g + 512].T).astype(BF)
        wo_t = wo_t.reshape(4, 128, 2, 512).transpose(1, 0, 2, 3)    # [128,4,2,512]
        wo_t = np.ascontiguousarray(wo_t).reshape(128, 4096)
        in_maps.append({
            "xt": xt, "wqk": wqk_t, "wv": wv_t, "wo": wo_t,
            "cos2": cos2, "sin2": sin2, "trimask": tri,
        })
    return in_maps


def kernel(x, W, Wo):
    from concourse.bass_utils import run_bass_kernel_spmd

    res = run_bass_kernel_spmd(_get_nc(), _in_maps(x, W, Wo),
                               list(range(NCORES)))
    out = np.empty((B, L, D), np.float32)
    for b in range(B):
        yb = (res.results[2 * b]["y"].astype(np.float32)
              + res.results[2 * b + 1]["y"].astype(np.float32))
        # [16, 2, 128, 512] blocks -> [L, D]
        out[b] = yb.transpose(0, 2, 1, 3).reshape(L, D)
    return out


def _install_ntff_hook_shim():
    """The trimmed repo lacks antenv.axon_hooks; reconstruct it so
    run_bass_kernel_spmd(trace=True) can NTFF-profile through axon."""
    import sys as _sys, types
    if "antenv.axon_hooks" in _sys.modules:
        return
    import antenv  # noqa: F401
    from trn_agent_boot.trn_boot import _ntff_profile_via_ctypes
    hook = _ntff_profile_via_ctypes("/opt/axon/libaxon_pjrt.so")
    mod = types.ModuleType("antenv.axon_hooks")
    mod.set_axon_ntff_profile_hook = lambda h: None
    mod.get_axon_ntff_profile_hook = lambda: hook
    _sys.modules["antenv.axon_hooks"] = mod


def kernel_traced(x, W, Wo, tmpdir=None):
    """Run with NTFF tracing; returns exec time ns (trace in tmpdir)."""
    from concourse.bass_utils import run_bass_kernel_spmd

    _install_ntff_hook_shim()
    res = run_bass_kernel_spmd(_get_nc(), _in_maps(x, W, Wo),
                               list(range(NCORES)), trace=True, tmpdir=tmpdir)
    return res.exec_time_ns


# revision 40
# speedup vs baseline: 1.0225x; 1.0130x over previous
"""Multi-head self-attention (B=4, L=2048, D=1024, H=16, RoPE, causal) on 8
Trainium2 NeuronCores.

Sharding: data-parallel over batch (4) x tensor-parallel over head groups (2).
Core i handles batch i//2, heads 8*(i%2) .. 8*(i%2)+8.  Each core computes its
QKV projection slice, RoPE, causal attention for its 8 heads, and a partial
output projection over its 512 d-columns; the host sums the two partials per
batch.

On-core dataflow (per core, matmul operands bf16, psum/softmax fp32):
  qkT[e,l] = Wqk_sub @ x^T       (e = 8 q-heads then 8 k-heads, dh-major)
  rope on qkT rows (pair-swap via stream_shuffle + bf16 cos/sin tables)
  V[l,e]   = x @ Wv_sub^T        stored with a ones column per head:
             vsb[k][:, 65h:65h+64] = V_h, vsb[k][:, 65h+64] = 1
  per head-pair p (heads 2p, 2p+1), per 512-wide q block:
    S^T[k,q] = K^T Q  (row-split pair, tile_position (0,0)/(64,0))
    P^T = exp(S^T/8), one contiguous ACT op + tri-mask per head
    av[0:64] / s[64] = [V_h | ones]^T @ P^T_h   (full-array, 65-col stationary
                       -> AV and the softmax denominator in one stream pass)
    norm: s rows staged to SBUF partition 64, 1/s in place on partition 64
    (DVE), broadcast across partitions via K=1 ones-matmuls at array row 64
    into PSUM, per-head muls on DVE
  y[l,e] partial = O^T.T @ Wo_sub  (lhsT = O^T chunks), bf16 output

Scheduling: demand-driven.  The straight-line head is only qk chunks c0/c4
l4=0 plus V0-3; every other qk chunk / V tile is woven into the attention
rounds as PE filler so the scalar engine (exp, the attention-phase pacer)
starts ~15us in and the PE never idles long enough for the HAM clock gate to
re-throttle mid-kernel.  Each round's normalization tail is deferred into the
next round.  Inputs are host-packed contiguous so every prologue DMA is one
descriptor per partition, ordered by first use.
"""
import sys
sys.path.insert(0, "/opt/trn_rl_repo")

import numpy as np
import ml_dtypes

B, L, D, H = 4, 2048, 1024, 16
DH = D // H  # 64
THETA = 100000.0
NCORES = 8
BF = ml_dtypes.bfloat16

_built = None


def _rope_tables():
    # [128, L] bf16: rows = 2 stacked heads' dh (64 each), identical per head.
    pos = np.arange(L, dtype=np.float32)
    inv_freq = (1.0 / THETA ** (np.arange(0, DH, 2, dtype=np.float32) / DH))
    ang = pos[None, :] * inv_freq[:, None]              # [32, L]
    cos = np.cos(ang)                                    # [32, L]
    sin = np.sin(ang)
    cos2 = np.repeat(cos, 2, axis=0)                     # rows 2p,2p+1 = cos_p
    sin2 = np.empty((DH, L), np.float32)
    sin2[0::2] = -sin
    sin2[1::2] = sin
    return (np.concatenate([cos2, cos2], 0).astype(BF),
            np.concatenate([sin2, sin2], 0).astype(BF))


def _build():
    import concourse.mybir as mybir
    import concourse.tile as tile
    from concourse import bacc

    FP32 = mybir.dt.float32
    BF16 = mybir.dt.bfloat16
    MUL = mybir.AluOpType.mult
    ADD = mybir.AluOpType.add
    EXP = mybir.ActivationFunctionType.Exp
    SWAP_MASK = [i ^ 1 for i in range(32)]

    nc = bacc.Bacc(None, target_bir_lowering=False)
    # host-packed layouts, fully contiguous per DMA
    xt_d = nc.declare_dram_parameter("xt", [4, 128, 4096], BF16, False)
    wqk_d = nc.declare_dram_parameter("wqk", [8, 128, 1024], BF16, False)
    wv_d = nc.declare_dram_parameter("wv", [128, 4096], BF16, False)
    wo_d = nc.declare_dram_parameter("wo", [128, 4096], BF16, False)
    cos_d = nc.declare_dram_parameter("cos2", [128, L], BF16, False)
    sin_d = nc.declare_dram_parameter("sin2", [128, L], BF16, False)
    tri_d = nc.declare_dram_parameter("trimask", [128, 128], BF16, False)
    # y stored as contiguous [t, 128, 1024] row-blocks == [L, D] row-major;
    # strided half-width writes halved the output-drain DMA bandwidth
    y_d = nc.declare_dram_parameter("y", [16, 128, 1024], BF16, True)

    with tile.TileContext(nc) as tc:
        import contextlib
        ctx = contextlib.ExitStack()
        with ctx:
            res = ctx.enter_context(tc.tile_pool(name="res", bufs=1))
            wq_pool = ctx.enter_context(tc.tile_pool(name="wqk", bufs=8))
            rope_pool = ctx.enter_context(tc.tile_pool(name="rope", bufs=4))
            pt_pool = ctx.enter_context(tc.tile_pool(name="pt", bufs=10))
            y_pool = ctx.enter_context(tc.tile_pool(name="yt", bufs=4))
            ytp_pool = ctx.enter_context(tc.tile_pool(name="ytp", bufs=2))

            xt_t = [res.tile([128, 4096], BF16, tag=f"xt{l4}", name=f"xt{l4}")
                    for l4 in range(4)]
            qkr = [res.tile([128, L], BF16, tag=f"qkr{c}", name=f"qkr{c}")
                   for c in range(8)]
            vsb = [res.tile([128, 520], BF16, tag=f"v{t}", name=f"v{t}")
                   for t in range(16)]
            wv_all = res.tile([128, 4096], BF16, tag="wv")
            wo_all = res.tile([128, 4096], BF16, tag="wo")
            cos_sb = res.tile([128, L], BF16, tag="cos")
            sin_sb = res.tile([128, L], BF16, tag="sin")
            tri_sb = res.tile([128, 128], BF16, tag="tri")
            nst = {n: res.tile([128, 512], FP32, tag=n, name=n)
                   for n in ("srowA", "srowB", "srow0A", "srow0B",
                             "avbl", "avs")}
            r0A = res.tile([128, 512], FP32, tag="r0A")
            r0B = res.tile([128, 512], FP32, tag="r0B")
            rbA = res.tile([128, 512], BF16, tag="rbA")
            rbB = res.tile([128, 512], BF16, tag="rbB")
            ones1 = res.tile([128, 64], BF16, tag="ones1")
            ot = [res.tile([128, L], BF16, tag=f"ot{p}", name=f"ot{p}")
                  for p in range(4)]

            def xta(d, sl):
                # global l-slice (within one 512 block) -> xt tile slice
                lt, q0 = divmod(sl.start, 512)
                n = sl.stop - sl.start
                assert q0 + n <= 512
                return xt_t[lt][:, 512 * d + q0:512 * d + q0 + n]

            # ones in every vsb slot; V copies overwrite cols 65h..65h+64
            for t in range(16):
                nc.vector.memset(vsb[t], 1.0)
            nc.vector.memset(ones1, 1.0)

            # ---- prologue DMAs, contiguous, ordered by first use ----
            wq = {}

            def load_w(c):
                w = wq_pool.tile([128, 1024], BF16, tag="w", name=f"w_{c}")
                nc.sync.dma_start(out=w, in_=wqk_d[c])
                wq[c] = w

            # The hardware runs up to 8 queued DMAs concurrently with
            # fair-shared HBM bandwidth, so the first compute-critical
            # transfers (w0, xt0 first half) would otherwise finish only
            # when the whole first batch does.  A tiny SBUF->SBUF dummy
            # DMA reading the critical tile stalls the queue until that
            # transfer completes, giving it the full bandwidth.
            prio = res.tile([1, 16], BF16, tag="prio")

            load_w(0)
            nc.sync.dma_start(out=xt_t[0][:, 0:2048], in_=xt_d[0][:, 0:2048])
            nc.sync.dma_start(out=prio, in_=xt_t[0][0:1, 0:16])
            nc.sync.dma_start(out=xt_t[0][:, 2048:4096],
                              in_=xt_d[0][:, 2048:4096])
            load_w(4)
            nc.sync.dma_start(out=cos_sb[:, 0:512], in_=cos_d[:, 0:512])
            nc.sync.dma_start(out=sin_sb[:, 0:512], in_=sin_d[:, 0:512])
            nc.sync.dma_start(out=prio, in_=xt_t[0][0:1, 2048:2064])
            nc.sync.dma_start(out=wv_all, in_=wv_d[:, :])
            nc.sync.dma_start(out=tri_sb, in_=tri_d[:, :])
            nc.sync.dma_start(out=prio, in_=wv_all[0:1, 0:16])
            load_w(1)
            load_w(5)
            nc.sync.dma_start(out=xt_t[1], in_=xt_d[1])
            load_w(2)
            load_w(6)
            load_w(3)
            load_w(7)
            nc.sync.dma_start(out=cos_sb[:, 512:2048], in_=cos_d[:, 512:2048])
            nc.sync.dma_start(out=sin_sb[:, 512:2048], in_=sin_d[:, 512:2048])
            nc.sync.dma_start(out=xt_t[2], in_=xt_d[2])
            nc.sync.dma_start(out=wo_all, in_=wo_d[:, :])
            nc.sync.dma_start(out=xt_t[3], in_=xt_d[3])

            # ---- emission helpers ----
            def emit_qk_l4(c, l4, qk_ps, tag="qkps"):
                w = wq[c]
                lsl = slice(512 * l4, 512 * l4 + 512)
                qkp = qk_ps.tile([128, 512], FP32, tag=tag)
                for d in range(8):
                    nc.tensor.matmul(qkp, w[:, 128 * d:128 * d + 128],
                                     xta(d, lsl), start=(d == 0),
                                     stop=(d == 7))
                shf = rope_pool.tile([128, 512], FP32, tag="shf")
                nc.vector.stream_shuffle(shf, qkp, SWAP_MASK)
                t1 = rope_pool.tile([128, 512], FP32, tag="t1")
                nc.vector.tensor_tensor(out=t1, in0=qkp, in1=cos_sb[:, lsl],
                                        op=MUL)
                t2 = rope_pool.tile([128, 512], FP32, tag="t2")
                # shf and sin are SBUF-resident, so this mul can run on
                # GpSimd, keeping the vector engine free for masks/norms
                nc.gpsimd.tensor_tensor(out=t2, in0=shf, in1=sin_sb[:, lsl],
                                        op=MUL)
                nc.gpsimd.tensor_tensor(out=qkr[c][:, lsl], in0=t1, in1=t2,
                                        op=ADD)

            def emit_v_tile(t, v_ps, tag="vps"):
                vp = v_ps.tile([128, 512], FP32, tag=tag)
                lsl = slice(128 * t, 128 * t + 128)
                for d in range(8):
                    nc.tensor.matmul(vp, xta(d, lsl),
                                     wv_all[:, 512 * d:512 * d + 512],
                                     start=(d == 0), stop=(d == 7))
                src = vp[:, :].rearrange("p (h e) -> p h e", h=8)
                dst = vsb[t][:, 0:520].rearrange("p (h e) -> p h e",
                                                 h=8, e=65)[:, :, 0:64]
                nc.scalar.copy(out=dst, in_=src)

            def emit_st_exp(jq, p, k, st_ps):
                qb0 = 512 * jq
                qt, kt = qkr[p], qkr[4 + p]
                kpos = 128 * k
                vs = max(0, kpos - qb0)
                st = st_ps.tile([128, 1024], FP32, tag="st")
                ksl = slice(kpos, kpos + 128)
                qsl = slice(qb0 + vs, qb0 + 512)
                nc.tensor.matmul(st[:, vs:512], kt[0:64, ksl], qt[0:64, qsl],
                                 start=True, stop=True, tile_position=(0, 0))
                nc.tensor.matmul(st[:, 512 + vs:1024], kt[64:128, ksl],
                                 qt[64:128, qsl], start=True, stop=True,
                                 tile_position=(64, 0))
                pt = pt_pool.tile([128, 1024], BF16, tag="pt")
                # strided AP: both heads' valid q ranges, skipping the
                # never-written [512:512+vs] junk between them
                src = st[:, :].rearrange("p (g q) -> p g q", g=2)[:, :, vs:512]
                dst = pt[:, :].rearrange("p (g q) -> p g q", g=2)[:, :, vs:512]
                nc.scalar.activation(out=dst, in_=src, func=EXP, scale=0.125)
                if kpos >= qb0:
                    dm = pt[:, :].rearrange("p (g q) -> p g q",
                                            g=2)[:, :, vs:vs + 128]
                    trib = tri_sb[:, :].unsqueeze(1).broadcast_to(
                        [128, 2, 128])
                    nc.vector.tensor_tensor(out=dm, in0=dm, in1=trib, op=MUL)
                return pt, vs

            def emit_av(p, k, nk, avA, avB, pt, vs):
                first, last = (k == 0), (k == nk - 1)
                isl = slice(vs, 512)
                ca = 130 * p
                nc.tensor.matmul(avA[0:65, isl], vsb[k][:, ca:ca + 65],
                                 pt[:, vs:512], start=first, stop=last)
                nc.tensor.matmul(avB[0:65, isl], vsb[k][:, ca + 65:ca + 130],
                                 pt[:, 512 + vs:1024], start=first, stop=last)

            def emit_norm(p, jq, avA, avB, rsb_ps, sceng=None,
                          on_group_done=None, avbl_eng=None):
                # PSUM is only DVE/ACT-readable; DVE cannot cross partition
                # offsets.  Stage s rows + av bulk off PSUM now (frees both av
                # banks), shift avB's partitions via SBUF->SBUF DMA, then in
                # the returned closure (fired a few ktiles into the NEXT
                # round) compute 1/s in place on partition 64 and BROADCAST it
                # across partitions with K=1 ones-matmuls at array row 64 into
                # a PSUM bank; per-head muls on DVE read that PSUM directly.
                qsl = slice(512 * jq, 512 * jq + 512)
                s = nst
                if sceng is None:
                    sceng = nc.scalar.copy
                sceng(out=s["srowA"][64:65, :], in_=avA[64:65, :])
                sceng(out=s["srowB"][64:65, :], in_=avB[64:65, :])
                nc.vector.tensor_copy(out=s["avs"][0:64, :],
                                      in_=avA[0:64, :])
                (avbl_eng or nc.vector.tensor_copy)(out=s["avbl"][0:64, :],
                                                    in_=avB[0:64, :])
                nc.sync.dma_start(out=s["srow0A"][0:1, :],
                                  in_=s["srowA"][64:65, :])
                nc.sync.dma_start(out=s["srow0B"][0:1, :],
                                  in_=s["srowB"][64:65, :])
                nc.sync.dma_start(out=s["avs"][64:128, :],
                                  in_=s["avbl"][0:64, :])

                def norm_head():
                    nc.vector.reciprocal_approx_fast(
                        out=r0A[0:1, :], in_=s["srow0A"][0:1, :])
                    nc.vector.reciprocal_approx_fast(
                        out=r0B[0:1, :], in_=s["srow0B"][0:1, :])
                    nc.vector.tensor_copy(out=rbA[0:1, :], in_=r0A[0:1, :])
                    nc.vector.tensor_copy(out=rbB[0:1, :], in_=r0B[0:1, :])
                    rp = rsb_ps.tile([128, 512], FP32, tag="rsb")
                    nc.tensor.matmul(rp[0:64, :], ones1[0:1, :],
                                     rbA[0:1, :], start=True, stop=True,
                                     tile_position=(0, 0))
                    nc.tensor.matmul(rp[64:128, :], ones1[0:1, :],
                                     rbB[0:1, :], start=True, stop=True,
                                     tile_position=(0, 64))

                    def norm_mul():
                        nc.vector.tensor_tensor(out=ot[p][:, qsl],
                                                in0=s["avs"][:, :],
                                                in1=rp[:, :], op=MUL)
                        # only once this write is emitted may the jq group's
                        # projection halves be queued (pops read ot[p])
                        if p == 3 and on_group_done is not None:
                            on_group_done(jq)
                    return norm_mul
                return norm_head

            def emit_proj_half(t, eh, y_ps, tag="yps", cast_eng=None):
                lsl = slice(128 * t, 128 * t + 128)
                yp = y_ps.tile([128, 512], FP32, tag=tag)
                for dc in range(4):
                    i = dc * 2 + eh
                    nc.tensor.matmul(yp, ot[dc][:, lsl],
                                     wo_all[:, 512 * i:512 * i + 512],
                                     start=(dc == 0), stop=(dc == 3))
                yt = y_pool.tile([128, 512], BF16, tag="yt")
                (cast_eng or nc.vector.tensor_copy)(out=yt, in_=yp)
                nc.sync.dma_start(out=y_d[t][:, 512 * eh:512 * eh + 512],
                                  in_=yt)

            pending_norm = [None]
            proj_queue = []

            def group_done(jq):
                proj_queue.extend(
                    (t, eh) for t in range(4 * jq, 4 * jq + 4)
                    for eh in range(2))

            def emit_round(jq, p, st_ps, av_ps, rsb_ps, skew, filler,
                           proj_cb=None, sceng=None, avbl_eng=None):
                """One attention round; pops one filler unit per ktile."""
                nk = 4 * (jq + 1)
                avA = av_ps.tile([128, 512], FP32, tag="avA",
                                 name=f"avA_{jq}_{p}")
                avB = av_ps.tile([128, 512], FP32, tag="avB",
                                 name=f"avB_{jq}_{p}")
                pending = []
                for k in range(nk):
                    pt, vs = emit_st_exp(jq, p, k, st_ps)
                    if k == 2 and pending_norm[0] is not None:
                        pending_norm[0] = pending_norm[0]()
                    elif k == 4 and pending_norm[0] is not None:
                        pending_norm[0]()
                        pending_norm[0] = None
                    if filler:
                        filler.pop(0)()
                    elif proj_cb is not None and k % 2 == 1:
                        proj_cb(1)
                    pending.append((k, pt, vs))
                    if len(pending) > skew:
                        kk, pp, vv = pending.pop(0)
                        emit_av(p, kk, nk, avA, avB, pp, vv)
                for kk, pp, vv in pending:
                    emit_av(p, kk, nk, avA, avB, pp, vv)
                if pending_norm[0] is not None:
                    pending_norm[0]()
                    pending_norm[0] = None
                pending_norm[0] = emit_norm(p, jq, avA, avB, rsb_ps,
                                            sceng, group_done, avbl_eng)

            # ---- phase 1: rounds jq=0,1 with qk/V units woven in ----
            rsb_ps = ctx.enter_context(
                tc.tile_pool(name="ps_rsb", bufs=1, space="PSUM"))
            with tc.tile_pool(name="ps_qk", bufs=2, space="PSUM") as qk_ps, \
                 tc.tile_pool(name="ps_v", bufs=1, space="PSUM") as v_ps, \
                 tc.tile_pool(name="ps_st1", bufs=1, space="PSUM") as st1_ps, \
                 tc.tile_pool(name="ps_av1", bufs=1, space="PSUM") as av1_ps:

                def qk_unit(c, l4):
                    return lambda: emit_qk_l4(c, l4, qk_ps)

                def v_unit(t):
                    return lambda: emit_v_tile(t, v_ps)

                def drain_proj1(n):
                    # group-0 proj halves reuse the V psum bank as filler;
                    # their output cast runs on the scalar engine, which has
                    # slack here, keeping DVE free for rope muls + masks
                    for _ in range(n):
                        if proj_queue:
                            emit_proj_half(*proj_queue.pop(0), v_ps,
                                           tag="vps",
                                           cast_eng=nc.scalar.copy)

                P1 = (lambda: drain_proj1(1))
                NULL = (lambda: None)

                # straight-line head: minimum needed to start round (0,0)
                emit_qk_l4(0, 0, qk_ps)
                emit_qk_l4(4, 0, qk_ps)
                for t in range(4):
                    emit_v_tile(t, v_ps)

                plan1 = [
                    (0, 0, [qk_unit(1, 0), qk_unit(5, 0), NULL, NULL]),
                    (0, 1, [qk_unit(2, 0), qk_unit(6, 0),
                            qk_unit(0, 1), qk_unit(4, 1)]),
                    (0, 2, [qk_unit(3, 0), qk_unit(7, 0),
                            qk_unit(1, 1), qk_unit(5, 1)]),
                    (0, 3, [qk_unit(2, 1), qk_unit(6, 1),
                            qk_unit(3, 1), qk_unit(7, 1)]),
                    (1, 0, [v_unit(4), v_unit(5), v_unit(6), v_unit(7),
                            qk_unit(0, 2), qk_unit(4, 2),
                            qk_unit(1, 2), qk_unit(5, 2)]),
                    (1, 1, [qk_unit(2, 2), qk_unit(6, 2),
                            qk_unit(3, 2), qk_unit(7, 2),
                            v_unit(8), v_unit(9),
                            qk_unit(0, 3), qk_unit(4, 3)]),
                ]
                for jq, p, fillers in plan1:
                    # srow + avbl staging on the scalar engine: phase-1 DVE
                    # is loaded with rope muls and masks, ACT has slack
                    emit_round(jq, p, st1_ps, av1_ps, rsb_ps, 2, fillers,
                               avbl_eng=nc.scalar.copy)

            # ---- phase 2: jq=2,3 rounds (minus (2,3)) + projection ----
            # late-needed qk l4=3 chunks, V12-15 and the proj halves are the
            # PE filler here so the PE stays busy (and the HAM clock gate
            # stays released) through the exp-paced big rounds.
            with tc.tile_pool(name="ps_st2", bufs=2, space="PSUM") as st2_ps, \
                 tc.tile_pool(name="ps_av2", bufs=1, space="PSUM") as av2_ps, \
                 tc.tile_pool(name="ps_aux", bufs=1, space="PSUM") as aux_ps:

                def drain_proj(n):
                    for _ in range(n):
                        if proj_queue:
                            emit_proj_half(*proj_queue.pop(0), aux_ps,
                                           tag="aux")

                def qk_unit2(c, l4):
                    return lambda: emit_qk_l4(c, l4, aux_ps, tag="aux")

                def v_unit2(t):
                    return lambda: emit_v_tile(t, aux_ps, tag="aux")

                P = (lambda: drain_proj(1))
                plan2 = [
                    (1, 2, [v_unit2(10), v_unit2(11),
                            qk_unit2(1, 3), qk_unit2(5, 3),
                            P, NULL, P, NULL]),
                    (1, 3, [qk_unit2(2, 3), qk_unit2(6, 3), P, NULL,
                            v_unit2(12), v_unit2(13), P, NULL]),
                    (2, 0, [qk_unit2(3, 3), qk_unit2(7, 3),
                            v_unit2(14), v_unit2(15), NULL, P, P, NULL,
                            P, NULL, NULL, NULL]),
                    (2, 1, [P, NULL, NULL, P, NULL, NULL, P, NULL,
                            NULL, P, NULL, NULL]),
                    (2, 2, [P, NULL, NULL, P, NULL, NULL, P, NULL,
                            NULL, NULL, NULL, NULL]),
                    (2, 3, [P, NULL, NULL, NULL, P, NULL, NULL, NULL,
                            P, NULL, NULL, NULL]),
                    (3, 0, [NULL, NULL, NULL, NULL, NULL, P, NULL, NULL,
                            P, NULL, NULL, P, NULL, NULL, NULL, NULL]),
                    (3, 1, [P, NULL, NULL, NULL, P, NULL, NULL, NULL,
                            P, NULL, NULL, NULL, NULL, NULL, NULL, NULL]),
                    (3, 2, [P, NULL, NULL, NULL, NULL, P, NULL, NULL,
                            NULL, P, NULL, NULL, NULL, NULL, NULL, NULL]),
                ]
                for jq, p, fillers in plan2:
                    emit_round(jq, p, st2_ps, av2_ps, rsb_ps, 2, fillers,
                               sceng=nc.vector.tensor_copy)

            # ---- phase 3: last round (3,3) ----
            with tc.tile_pool(name="ps_st3", bufs=1, space="PSUM") as st3_ps, \
                 tc.tile_pool(name="ps_av3", bufs=1, space="PSUM") as av3_ps, \
                 tc.tile_pool(name="ps_y3", bufs=1, space="PSUM") as y3_ps:

                def drain_proj3(n):
                    for _ in range(n):
                        if proj_queue:
                            emit_proj_half(*proj_queue.pop(0), y3_ps)

                P3 = (lambda: drain_proj3(1))
                fillers3 = ([NULL] * 4 + [P3] + [NULL] * 11)
                emit_round(3, 3, st3_ps, av3_ps, rsb_ps, 2, fillers3,
                           sceng=nc.scalar.copy, avbl_eng=nc.scalar.copy)

            # ---- tail.  st/av/y3 pools are closed, freeing 7 PSUM banks.
            # Two leftover group-2 fulls plus six group-3 dc=0..2 partial
            # accumulations run while the final norm chain executes, keeping
            # the PE busy and the HAM clock released; after the norm each
            # partial needs only its single dc=3 matmul. ----
            with tc.tile_pool(name="ps_y4", bufs=7, space="PSUM") as y4_ps:
                lead = []
                while proj_queue and len(lead) < 2:
                    lead.append(proj_queue.pop(0))
                for t, eh in lead:
                    emit_proj_half(t, eh, y4_ps, tag="y4")
                PART = [(12, 0), (12, 1), (13, 0), (13, 1), (14, 0), (14, 1)]
                parts = []
                for t, eh in PART:
                    yp = y4_ps.tile([128, 512], FP32, tag="y4",
                                    name=f"ypre{t}_{eh}")
                    for dc in range(3):
                        i = dc * 2 + eh
                        nc.tensor.matmul(yp, ot[dc][:, 128 * t:128 * t + 128],
                                         wo_all[:, 512 * i:512 * i + 512],
                                         start=(dc == 0), stop=False)
                    parts.append((t, eh, yp))
                nh_mul = pending_norm[0]()   # recips/casts + rsb matmuls
                pending_norm[0] = None
                nh_mul()                     # final ot write; queues group 3
                # finish partials in (eh=0, eh=1) pairs: both casts land in
                # one [128, 1024] tile so the drain DMA gets 2KB lines
                for t, eh, yp in parts:
                    i = 3 * 2 + eh
                    nc.tensor.matmul(yp, ot[3][:, 128 * t:128 * t + 128],
                                     wo_all[:, 512 * i:512 * i + 512],
                                     start=False, stop=True)
                for k4 in range(0, len(parts), 2):
                    (t, _, ypA), (_, _, ypB) = parts[k4], parts[k4 + 1]
                    ytp = ytp_pool.tile([128, 1024], BF16, tag="ytp")
                    nc.vector.tensor_copy(out=ytp[:, 0:512], in_=ypA)
                    nc.scalar.copy(out=ytp[:, 512:1024], in_=ypB)
                    nc.sync.dma_start(out=y_d[t], in_=ytp)
                rest = [x for x in proj_queue if x not in PART]
                for k4 in range(0, len(rest), 2):
                    if (k4 + 1 < len(rest) and rest[k4][0] == rest[k4 + 1][0]
                            and rest[k4][1] == 0):
                        t = rest[k4][0]
                        yps = []
                        for eh in (0, 1):
                            yp = y4_ps.tile([128, 512], FP32, tag="y4")
                            for dc in range(4):
                                i = dc * 2 + eh
                                nc.tensor.matmul(
                                    yp, ot[dc][:, 128 * t:128 * t + 128],
                                    wo_all[:, 512 * i:512 * i + 512],
                                    start=(dc == 0), stop=(dc == 3))
                            yps.append(yp)
                        ytp = ytp_pool.tile([128, 1024], BF16, tag="ytp")
                        nc.vector.tensor_copy(out=ytp[:, 0:512], in_=yps[0])
                        nc.scalar.copy(out=ytp[:, 512:1024], in_=yps[1])
                        nc.sync.dma_start(out=y_d[t], in_=ytp)
                    else:
                        emit_proj_half(*rest[k4], y4_ps, tag="y4")
                        if k4 + 1 < len(rest):
                            emit_proj_half(*rest[k4 + 1], y4_ps, tag="y4",
                                           cast_eng=nc.scalar.copy)
    nc.compile()
    return nc


def _get_nc():
    global _built
    if _built is None:
        _built = _build()
    return _built


def _in_maps(x, W, Wo):
    x = np.asarray(x, np.float32)
    W = np.asarray(W, np.float32)
    Wo = np.asarray(Wo, np.float32)

    cos2, sin2 = _rope_tables()
    tri = np.zeros((128, 128), np.float32)
    p_idx = np.arange(128)
    tri[p_idx[:, None] <= p_idx[None, :]] = 1.0  # valid: k <= q
    tri = tri.astype(BF)

    in_maps = []
    for core in range(NCORES):
        b, g = core // 2, core % 2
        xt = np.ascontiguousarray(x[b].T).astype(BF)                 # [D, L]
        # -> [l4, 128(d-part), 8(dchunk) x 512] contiguous
        xt = xt.reshape(8, 128, 4, 512).transpose(2, 1, 0, 3)
        xt = np.ascontiguousarray(xt).reshape(4, 128, 4096)
        wq = W[512 * g:512 * g + 512]                                # [512, D]
        wk = W[D + 512 * g:D + 512 * g + 512]
        wv = W[2 * D + 512 * g:2 * D + 512 * g + 512]
        wqk_t = np.ascontiguousarray(
            np.concatenate([wq, wk], 0).T).astype(BF)                # [D, 1024]
        # -> [echunk, 128(d-part), 1024(8 dchunk x 128 e)]
        wqk_t = wqk_t.reshape(8, 128, 8, 128).transpose(2, 1, 0, 3)
        wqk_t = np.ascontiguousarray(wqk_t.reshape(8, 128, 1024))
        wv_t = np.ascontiguousarray(wv.T).astype(BF)                 # [D, 512]
        wv_t = np.ascontiguousarray(
            wv_t.reshape(8, 128, 512).transpose(1, 0, 2))            # [128,8,512]
        wv_t = wv_t.reshape(128, 4096)
        wo_t = np.ascontiguousarray(Wo[:, 512 * g:512 * g + 512].T).astype(BF)
        wo_t = wo_t.reshape(4, 128, 2, 512).transpose(1, 0, 2, 3)    # [128,4,2,512]
        wo_t = np.ascontiguousarray(wo_t).reshape(128, 4096)
        in_maps.append({
            "xt": xt, "wqk": wqk_t, "wv": wv_t, "wo": wo_t,
            "cos2": cos2, "sin2": sin2, "trimask": tri,
        })
    return in_maps


def kernel(x, W, Wo):
    from concourse.bass_utils import run_bass_kernel_spmd

    res = run_bass_kernel_spmd(_get_nc(), _in_maps(x, W, Wo),
                               list(range(NCORES)))
    out = np.empty((B, L, D), np.float32)
    for b in range(B):
        yb = (res.results[2 * b]["y"].astype(np.float32)
              + res.results[2 * b + 1]["y"].astype(np.float32))
        out[b] = yb.reshape(L, D)
    return out


def _install_ntff_hook_shim():
    """The trimmed repo lacks antenv.axon_hooks; reconstruct it so
    run_bass_kernel_spmd(trace=True) can NTFF-profile through axon."""
    import sys as _sys, types
    if "antenv.axon_hooks" in _sys.modules:
        return
    import antenv  # noqa: F401
    from trn_agent_boot.trn_boot import _ntff_profile_via_ctypes
    hook = _ntff_profile_via_ctypes("/opt/axon/libaxon_pjrt.so")
    mod = types.ModuleType("antenv.axon_hooks")
    mod.set_axon_ntff_profile_hook = lambda h: None
    mod.get_axon_ntff_profile_hook = lambda: hook
    _sys.modules["antenv.axon_hooks"] = mod


def kernel_traced(x, W, Wo, tmpdir=None):
    """Run with NTFF tracing; returns exec time ns (trace in tmpdir)."""
    from concourse.bass_utils import run_bass_kernel_spmd

    _install_ntff_hook_shim()
    res = run_bass_kernel_spmd(_get_nc(), _in_maps(x, W, Wo),
                               list(range(NCORES)), trace=True, tmpdir=tmpdir)
    return res.exec_time_ns


# revision 43
# speedup vs baseline: 1.0324x; 1.0097x over previous
"""Multi-head self-attention (B=4, L=2048, D=1024, H=16, RoPE, causal) on 8
Trainium2 NeuronCores.

Sharding: data-parallel over batch (4) x tensor-parallel over head groups (2).
Core i handles batch i//2, heads 8*(i%2) .. 8*(i%2)+8.  Each core computes its
QKV projection slice, RoPE, causal attention for its 8 heads, and a partial
output projection over its 512 d-columns; the host sums the two partials per
batch.

On-core dataflow (per core, matmul operands bf16, psum/softmax fp32):
  qkT[e,l] = Wqk_sub @ x^T       (e = 8 q-heads then 8 k-heads, dh-major)
  rope on qkT rows (pair-swap via stream_shuffle + bf16 cos/sin tables)
  V[l,e]   = x @ Wv_sub^T        stored with a ones column per head:
             vsb[k][:, 65h:65h+64] = V_h, vsb[k][:, 65h+64] = 1
  per head-pair p (heads 2p, 2p+1), per 512-wide q block:
    S^T[k,q] = K^T Q  (row-split pair, tile_position (0,0)/(64,0))
    P^T = exp(S^T/8), one contiguous ACT op + tri-mask per head
    av[0:64] / s[64] = [V_h | ones]^T @ P^T_h   (full-array, 65-col stationary
                       -> AV and the softmax denominator in one stream pass)
    norm: s rows staged to SBUF partition 64, 1/s in place on partition 64
    (DVE), broadcast across partitions via K=1 ones-matmuls at array row 64
    into PSUM, per-head muls on DVE
  y[l,e] partial = O^T.T @ Wo_sub  (lhsT = O^T chunks), bf16 output

Scheduling: demand-driven.  The straight-line head is only qk chunks c0/c4
l4=0 plus V0-3; every other qk chunk / V tile is woven into the attention
rounds as PE filler so the scalar engine (exp, the attention-phase pacer)
starts ~15us in and the PE never idles long enough for the HAM clock gate to
re-throttle mid-kernel.  Each round's normalization tail is deferred into the
next round.  Inputs are host-packed contiguous so every prologue DMA is one
descriptor per partition, ordered by first use.
"""
import sys
sys.path.insert(0, "/opt/trn_rl_repo")

import numpy as np
import ml_dtypes

B, L, D, H = 4, 2048, 1024, 16
DH = D // H  # 64
THETA = 100000.0
NCORES = 8
BF = ml_dtypes.bfloat16

_built = None


def _rope_tables():
    # [128, L] bf16: rows = 2 stacked heads' dh (64 each), identical per head.
    pos = np.arange(L, dtype=np.float32)
    inv_freq = (1.0 / THETA ** (np.arange(0, DH, 2, dtype=np.float32) / DH))
    ang = pos[None, :] * inv_freq[:, None]              # [32, L]
    cos = np.cos(ang)                                    # [32, L]
    sin = np.sin(ang)
    cos2 = np.repeat(cos, 2, axis=0)                     # rows 2p,2p+1 = cos_p
    sin2 = np.empty((DH, L), np.float32)
    sin2[0::2] = -sin
    sin2[1::2] = sin
    return (np.concatenate([cos2, cos2], 0).astype(BF),
            np.concatenate([sin2, sin2], 0).astype(BF))


def _build():
    import concourse.mybir as mybir
    import concourse.tile as tile
    from concourse import bacc

    FP32 = mybir.dt.float32
    BF16 = mybir.dt.bfloat16
    MUL = mybir.AluOpType.mult
    ADD = mybir.AluOpType.add
    EXP = mybir.ActivationFunctionType.Exp
    SWAP_MASK = [i ^ 1 for i in range(32)]

    nc = bacc.Bacc(None, target_bir_lowering=False)
    # host-packed layouts, fully contiguous per DMA
    xt_d = nc.declare_dram_parameter("xt", [4, 128, 4096], BF16, False)
    wqk_d = nc.declare_dram_parameter("wqk", [8, 128, 1024], BF16, False)
    wv_d = nc.declare_dram_parameter("wv", [128, 4096], BF16, False)
    wo_d = nc.declare_dram_parameter("wo", [128, 4096], BF16, False)
    cos_d = nc.declare_dram_parameter("cos2", [128, L], BF16, False)
    sin_d = nc.declare_dram_parameter("sin2", [128, L], BF16, False)
    tri_d = nc.declare_dram_parameter("trimask", [128, 128], BF16, False)
    # y stored as contiguous [t, 128, 1024] row-blocks == [L, D] row-major;
    # strided half-width writes halved the output-drain DMA bandwidth
    y_d = nc.declare_dram_parameter("y", [16, 128, 1024], BF16, True)

    with tile.TileContext(nc) as tc:
        import contextlib
        ctx = contextlib.ExitStack()
        with ctx:
            res = ctx.enter_context(tc.tile_pool(name="res", bufs=1))
            wq_pool = ctx.enter_context(tc.tile_pool(name="wqk", bufs=8))
            rope_pool = ctx.enter_context(tc.tile_pool(name="rope", bufs=4))
            pt_pool = ctx.enter_context(tc.tile_pool(name="pt", bufs=10))
            y_pool = ctx.enter_context(tc.tile_pool(name="yt", bufs=4))
            ytp_pool = ctx.enter_context(tc.tile_pool(name="ytp", bufs=2))

            xt_t = [res.tile([128, 4096], BF16, tag=f"xt{l4}", name=f"xt{l4}")
                    for l4 in range(4)]
            qkr = [res.tile([128, L], BF16, tag=f"qkr{c}", name=f"qkr{c}")
                   for c in range(8)]
            vsb = [res.tile([128, 520], BF16, tag=f"v{t}", name=f"v{t}")
                   for t in range(16)]
            wv_all = res.tile([128, 4096], BF16, tag="wv")
            wo_all = res.tile([128, 4096], BF16, tag="wo")
            cos_sb = res.tile([128, L], BF16, tag="cos")
            sin_sb = res.tile([128, L], BF16, tag="sin")
            tri_sb = res.tile([128, 128], BF16, tag="tri")
            nst = {n: res.tile([128, 512], FP32, tag=n, name=n)
                   for n in ("srowA", "srowB", "srow0A", "srow0B",
                             "avbl", "avs")}
            r0A = res.tile([128, 512], FP32, tag="r0A")
            r0B = res.tile([128, 512], FP32, tag="r0B")
            rbA = res.tile([128, 512], BF16, tag="rbA")
            rbB = res.tile([128, 512], BF16, tag="rbB")
            ones1 = res.tile([128, 64], BF16, tag="ones1")
            ot = [res.tile([128, L], BF16, tag=f"ot{p}", name=f"ot{p}")
                  for p in range(4)]

            def xta(d, sl):
                # global l-slice (within one 512 block) -> xt tile slice
                lt, q0 = divmod(sl.start, 512)
                n = sl.stop - sl.start
                assert q0 + n <= 512
                return xt_t[lt][:, 512 * d + q0:512 * d + q0 + n]

            # ones in every vsb slot; V copies overwrite cols 65h..65h+64
            for t in range(16):
                nc.vector.memset(vsb[t], 1.0)
            nc.vector.memset(ones1, 1.0)

            # ---- prologue DMAs, contiguous, ordered by first use ----
            wq = {}

            def load_w(c):
                w = wq_pool.tile([128, 1024], BF16, tag="w", name=f"w_{c}")
                nc.sync.dma_start(out=w, in_=wqk_d[c])
                wq[c] = w

            # The hardware runs up to 8 queued DMAs concurrently with
            # fair-shared HBM bandwidth, so the first compute-critical
            # transfers (w0, xt0 first half) would otherwise finish only
            # when the whole first batch does.  A tiny SBUF->SBUF dummy
            # DMA reading the critical tile stalls the queue until that
            # transfer completes, giving it the full bandwidth.
            prio = res.tile([1, 16], BF16, tag="prio")

            load_w(0)
            nc.sync.dma_start(out=xt_t[0][:, 0:2048], in_=xt_d[0][:, 0:2048])
            nc.sync.dma_start(out=prio, in_=xt_t[0][0:1, 0:16])
            nc.sync.dma_start(out=xt_t[0][:, 2048:4096],
                              in_=xt_d[0][:, 2048:4096])
            load_w(4)
            nc.sync.dma_start(out=cos_sb[:, 0:512], in_=cos_d[:, 0:512])
            nc.sync.dma_start(out=sin_sb[:, 0:512], in_=sin_d[:, 0:512])
            nc.sync.dma_start(out=prio, in_=xt_t[0][0:1, 2048:2064])
            nc.sync.dma_start(out=wv_all, in_=wv_d[:, :])
            nc.sync.dma_start(out=tri_sb, in_=tri_d[:, :])
            nc.sync.dma_start(out=prio, in_=wv_all[0:1, 0:16])
            load_w(1)
            load_w(5)
            nc.sync.dma_start(out=xt_t[1], in_=xt_d[1])
            load_w(2)
            load_w(6)
            load_w(3)
            load_w(7)
            nc.sync.dma_start(out=cos_sb[:, 512:2048], in_=cos_d[:, 512:2048])
            nc.sync.dma_start(out=sin_sb[:, 512:2048], in_=sin_d[:, 512:2048])
            nc.sync.dma_start(out=xt_t[2], in_=xt_d[2])
            nc.sync.dma_start(out=wo_all, in_=wo_d[:, :])
            nc.sync.dma_start(out=xt_t[3], in_=xt_d[3])

            # ---- emission helpers ----
            def emit_qk_l4(c, l4, qk_ps, tag="qkps"):
                w = wq[c]
                lsl = slice(512 * l4, 512 * l4 + 512)
                qkp = qk_ps.tile([128, 512], FP32, tag=tag)
                for d in range(8):
                    nc.tensor.matmul(qkp, w[:, 128 * d:128 * d + 128],
                                     xta(d, lsl), start=(d == 0),
                                     stop=(d == 7))
                shf = rope_pool.tile([128, 512], FP32, tag="shf")
                nc.vector.stream_shuffle(shf, qkp, SWAP_MASK)
                t1 = rope_pool.tile([128, 512], FP32, tag="t1")
                nc.vector.tensor_tensor(out=t1, in0=qkp, in1=cos_sb[:, lsl],
                                        op=MUL)
                t2 = rope_pool.tile([128, 512], FP32, tag="t2")
                # shf and sin are SBUF-resident, so this mul can run on
                # GpSimd, keeping the vector engine free for masks/norms
                nc.gpsimd.tensor_tensor(out=t2, in0=shf, in1=sin_sb[:, lsl],
                                        op=MUL)
                nc.gpsimd.tensor_tensor(out=qkr[c][:, lsl], in0=t1, in1=t2,
                                        op=ADD)

            def emit_v_tile(t, v_ps, tag="vps"):
                vp = v_ps.tile([128, 512], FP32, tag=tag)
                lsl = slice(128 * t, 128 * t + 128)
                for d in range(8):
                    nc.tensor.matmul(vp, xta(d, lsl),
                                     wv_all[:, 512 * d:512 * d + 512],
                                     start=(d == 0), stop=(d == 7))
                src = vp[:, :].rearrange("p (h e) -> p h e", h=8)
                dst = vsb[t][:, 0:520].rearrange("p (h e) -> p h e",
                                                 h=8, e=65)[:, :, 0:64]
                nc.scalar.copy(out=dst, in_=src)

            def emit_st_exp(jq, p, k, st_ps):
                qb0 = 512 * jq
                qt, kt = qkr[p], qkr[4 + p]
                kpos = 128 * k
                vs = max(0, kpos - qb0)
                st = st_ps.tile([128, 1024], FP32, tag="st")
                ksl = slice(kpos, kpos + 128)
                qsl = slice(qb0 + vs, qb0 + 512)
                nc.tensor.matmul(st[:, vs:512], kt[0:64, ksl], qt[0:64, qsl],
                                 start=True, stop=True, tile_position=(0, 0))
                nc.tensor.matmul(st[:, 512 + vs:1024], kt[64:128, ksl],
                                 qt[64:128, qsl], start=True, stop=True,
                                 tile_position=(64, 0))
                pt = pt_pool.tile([128, 1024], BF16, tag="pt")
                # strided AP: both heads' valid q ranges, skipping the
                # never-written [512:512+vs] junk between them
                src = st[:, :].rearrange("p (g q) -> p g q", g=2)[:, :, vs:512]
                dst = pt[:, :].rearrange("p (g q) -> p g q", g=2)[:, :, vs:512]
                nc.scalar.activation(out=dst, in_=src, func=EXP, scale=0.125)
                if kpos >= qb0:
                    dm = pt[:, :].rearrange("p (g q) -> p g q",
                                            g=2)[:, :, vs:vs + 128]
                    trib = tri_sb[:, :].unsqueeze(1).broadcast_to(
                        [128, 2, 128])
                    nc.vector.tensor_tensor(out=dm, in0=dm, in1=trib, op=MUL)
                return pt, vs

            def emit_av(p, k, nk, avA, avB, pt, vs):
                first, last = (k == 0), (k == nk - 1)
                isl = slice(vs, 512)
                ca = 130 * p
                nc.tensor.matmul(avA[0:65, isl], vsb[k][:, ca:ca + 65],
                                 pt[:, vs:512], start=first, stop=last)
                nc.tensor.matmul(avB[0:65, isl], vsb[k][:, ca + 65:ca + 130],
                                 pt[:, 512 + vs:1024], start=first, stop=last)

            def emit_norm(p, jq, avA, avB, rsb_ps, sceng=None,
                          on_group_done=None, avbl_eng=None):
                # PSUM is only DVE/ACT-readable; DVE cannot cross partition
                # offsets.  Stage s rows + av bulk off PSUM now (frees both av
                # banks), shift avB's partitions via SBUF->SBUF DMA, then in
                # the returned closure (fired a few ktiles into the NEXT
                # round) compute 1/s in place on partition 64 and BROADCAST it
                # across partitions with K=1 ones-matmuls at array row 64 into
                # a PSUM bank; per-head muls on DVE read that PSUM directly.
                qsl = slice(512 * jq, 512 * jq + 512)
                s = nst
                if sceng is None:
                    sceng = nc.scalar.copy
                sceng(out=s["srowA"][64:65, :], in_=avA[64:65, :])
                sceng(out=s["srowB"][64:65, :], in_=avB[64:65, :])
                nc.vector.tensor_copy(out=s["avs"][0:64, :],
                                      in_=avA[0:64, :])
                (avbl_eng or nc.vector.tensor_copy)(out=s["avbl"][0:64, :],
                                                    in_=avB[0:64, :])
                nc.sync.dma_start(out=s["srow0A"][0:1, :],
                                  in_=s["srowA"][64:65, :])
                nc.sync.dma_start(out=s["srow0B"][0:1, :],
                                  in_=s["srowB"][64:65, :])
                nc.sync.dma_start(out=s["avs"][64:128, :],
                                  in_=s["avbl"][0:64, :])

                def norm_head():
                    nc.vector.reciprocal_approx_fast(
                        out=r0A[0:1, :], in_=s["srow0A"][0:1, :])
                    nc.vector.reciprocal_approx_fast(
                        out=r0B[0:1, :], in_=s["srow0B"][0:1, :])
                    nc.vector.tensor_copy(out=rbA[0:1, :], in_=r0A[0:1, :])
                    nc.vector.tensor_copy(out=rbB[0:1, :], in_=r0B[0:1, :])
                    rp = rsb_ps.tile([128, 512], FP32, tag="rsb")
                    nc.tensor.matmul(rp[0:64, :], ones1[0:1, :],
                                     rbA[0:1, :], start=True, stop=True,
                                     tile_position=(0, 0))
                    nc.tensor.matmul(rp[64:128, :], ones1[0:1, :],
                                     rbB[0:1, :], start=True, stop=True,
                                     tile_position=(0, 64))

                    def norm_mul():
                        nc.vector.tensor_tensor(out=ot[p][:, qsl],
                                                in0=s["avs"][:, :],
                                                in1=rp[:, :], op=MUL)
                        # only once this write is emitted may the jq group's
                        # projection halves be queued (pops read ot[p])
                        if p == 3 and on_group_done is not None:
                            on_group_done(jq)
                    return norm_mul
                return norm_head

            def emit_proj_half(t, eh, y_ps, tag="yps", cast_eng=None):
                lsl = slice(128 * t, 128 * t + 128)
                yp = y_ps.tile([128, 512], FP32, tag=tag)
                for dc in range(4):
                    i = dc * 2 + eh
                    nc.tensor.matmul(yp, ot[dc][:, lsl],
                                     wo_all[:, 512 * i:512 * i + 512],
                                     start=(dc == 0), stop=(dc == 3))
                yt = y_pool.tile([128, 512], BF16, tag="yt")
                (cast_eng or nc.vector.tensor_copy)(out=yt, in_=yp)
                nc.sync.dma_start(out=y_d[t][:, 512 * eh:512 * eh + 512],
                                  in_=yt)

            pending_norm = [None]
            proj_queue = []

            def group_done(jq):
                proj_queue.extend(
                    (t, eh) for t in range(4 * jq, 4 * jq + 4)
                    for eh in range(2))

            def emit_round(jq, p, st_ps, av_ps, rsb_ps, skew, filler,
                           proj_cb=None, sceng=None, avbl_eng=None):
                """One attention round; pops one filler unit per ktile."""
                nk = 4 * (jq + 1)
                avA = av_ps.tile([128, 512], FP32, tag="avA",
                                 name=f"avA_{jq}_{p}")
                avB = av_ps.tile([128, 512], FP32, tag="avB",
                                 name=f"avB_{jq}_{p}")
                pending = []
                for k in range(nk):
                    pt, vs = emit_st_exp(jq, p, k, st_ps)
                    if k == 2 and pending_norm[0] is not None:
                        pending_norm[0] = pending_norm[0]()
                    elif k == 4 and pending_norm[0] is not None:
                        pending_norm[0]()
                        pending_norm[0] = None
                    if filler:
                        filler.pop(0)()
                    elif proj_cb is not None and k % 2 == 1:
                        proj_cb(1)
                    pending.append((k, pt, vs))
                    if len(pending) > skew:
                        kk, pp, vv = pending.pop(0)
                        emit_av(p, kk, nk, avA, avB, pp, vv)
                for kk, pp, vv in pending:
                    emit_av(p, kk, nk, avA, avB, pp, vv)
                if pending_norm[0] is not None:
                    pending_norm[0]()
                    pending_norm[0] = None
                pending_norm[0] = emit_norm(p, jq, avA, avB, rsb_ps,
                                            sceng, group_done, avbl_eng)

            # ---- phase 1: rounds jq=0,1 with qk/V units woven in ----
            rsb_ps = ctx.enter_context(
                tc.tile_pool(name="ps_rsb", bufs=1, space="PSUM"))
            with tc.tile_pool(name="ps_qk", bufs=2, space="PSUM") as qk_ps, \
                 tc.tile_pool(name="ps_v", bufs=1, space="PSUM") as v_ps, \
                 tc.tile_pool(name="ps_st1", bufs=1, space="PSUM") as st1_ps, \
                 tc.tile_pool(name="ps_av1", bufs=1, space="PSUM") as av1_ps:

                def qk_unit(c, l4):
                    return lambda: emit_qk_l4(c, l4, qk_ps)

                def v_unit(t):
                    return lambda: emit_v_tile(t, v_ps)

                def drain_proj1(n):
                    # group-0 proj halves reuse the V psum bank as filler;
                    # their output cast runs on the scalar engine, which has
                    # slack here, keeping DVE free for rope muls + masks
                    for _ in range(n):
                        if proj_queue:
                            emit_proj_half(*proj_queue.pop(0), v_ps,
                                           tag="vps",
                                           cast_eng=nc.scalar.copy)

                P1 = (lambda: drain_proj1(1))
                NULL = (lambda: None)

                # straight-line head: minimum needed to start round (0,0)
                emit_qk_l4(0, 0, qk_ps)
                emit_qk_l4(4, 0, qk_ps)
                for t in range(4):
                    emit_v_tile(t, v_ps)

                plan1 = [
                    (0, 0, [qk_unit(1, 0), qk_unit(5, 0), NULL, NULL]),
                    (0, 1, [qk_unit(2, 0), qk_unit(6, 0),
                            qk_unit(0, 1), qk_unit(4, 1)]),
                    (0, 2, [qk_unit(3, 0), qk_unit(7, 0),
                            qk_unit(1, 1), qk_unit(5, 1)]),
                    (0, 3, [qk_unit(2, 1), qk_unit(6, 1),
                            qk_unit(3, 1), qk_unit(7, 1)]),
                    (1, 0, [v_unit(4), v_unit(5), v_unit(6), v_unit(7),
                            qk_unit(0, 2), qk_unit(4, 2),
                            qk_unit(1, 2), qk_unit(5, 2)]),
                    (1, 1, [qk_unit(2, 2), qk_unit(6, 2),
                            qk_unit(3, 2), qk_unit(7, 2),
                            v_unit(8), v_unit(9),
                            qk_unit(0, 3), P1]),
                ]
                for jq, p, fillers in plan1:
                    # srow + avbl staging on the scalar engine: phase-1 DVE
                    # is loaded with rope muls and masks, ACT has slack
                    emit_round(jq, p, st1_ps, av1_ps, rsb_ps, 2, fillers,
                               avbl_eng=nc.scalar.copy)

            # ---- phase 2: jq=2,3 rounds (minus (2,3)) + projection ----
            # late-needed qk l4=3 chunks, V12-15 and the proj halves are the
            # PE filler here so the PE stays busy (and the HAM clock gate
            # stays released) through the exp-paced big rounds.
            with tc.tile_pool(name="ps_st2", bufs=2, space="PSUM") as st2_ps, \
                 tc.tile_pool(name="ps_av2", bufs=1, space="PSUM") as av2_ps, \
                 tc.tile_pool(name="ps_aux", bufs=1, space="PSUM") as aux_ps:

                def drain_proj(n):
                    for _ in range(n):
                        if proj_queue:
                            emit_proj_half(*proj_queue.pop(0), aux_ps,
                                           tag="aux")

                def qk_unit2(c, l4):
                    return lambda: emit_qk_l4(c, l4, aux_ps, tag="aux")

                def v_unit2(t):
                    return lambda: emit_v_tile(t, aux_ps, tag="aux")

                P = (lambda: drain_proj(1))
                plan2 = [
                    (1, 2, [v_unit2(10), v_unit2(11), P, P,
                            P, NULL, P, NULL]),
                    (1, 3, [P, NULL, P, NULL, P, NULL, NULL, NULL]),
                    (2, 0, [NULL, NULL, NULL, NULL, NULL, P, P, NULL,
                            P, P, NULL, NULL]),
                    (2, 1, [P, NULL, NULL, P, NULL, NULL, P, NULL,
                            NULL, P, NULL, NULL]),
                    (2, 2, [v_unit2(12), v_unit2(13), P, NULL, NULL, NULL,
                            P, NULL, NULL, NULL, NULL, NULL]),
                    (2, 3, [v_unit2(14), v_unit2(15), P, NULL, NULL, NULL,
                            P, NULL, NULL, NULL, NULL, NULL]),
                    (3, 0, [qk_unit2(4, 3), qk_unit2(1, 3), NULL, NULL,
                            NULL, P, NULL, NULL, P, NULL, NULL, P,
                            NULL, NULL, NULL, NULL]),
                    (3, 1, [qk_unit2(5, 3), qk_unit2(2, 3), NULL, NULL,
                            P, NULL, NULL, NULL, P, NULL, NULL, NULL,
                            NULL, NULL, NULL, NULL]),
                    (3, 2, [qk_unit2(6, 3), qk_unit2(3, 3), NULL, NULL,
                            P, NULL, NULL, NULL, P, NULL, NULL, NULL,
                            NULL, NULL, NULL, NULL]),
                ]
                for jq, p, fillers in plan2:
                    emit_round(jq, p, st2_ps, av2_ps, rsb_ps, 2, fillers,
                               sceng=nc.vector.tensor_copy)

            # ---- phase 3: last round (3,3) ----
            with tc.tile_pool(name="ps_st3", bufs=1, space="PSUM") as st3_ps, \
                 tc.tile_pool(name="ps_av3", bufs=1, space="PSUM") as av3_ps, \
                 tc.tile_pool(name="ps_y3", bufs=1, space="PSUM") as y3_ps:

                def drain_proj3(n):
                    for _ in range(n):
                        if proj_queue:
                            emit_proj_half(*proj_queue.pop(0), y3_ps)

                P3 = (lambda: drain_proj3(1))

                def qk_unit3(c, l4):
                    return lambda: emit_qk_l4(c, l4, y3_ps, tag="yps")

                fillers3 = ([qk_unit3(7, 3), NULL, NULL, NULL, P3]
                            + [NULL] * 11)
                emit_round(3, 3, st3_ps, av3_ps, rsb_ps, 2, fillers3,
                           sceng=nc.scalar.copy, avbl_eng=nc.scalar.copy)

            # ---- tail.  st/av/y3 pools are closed, freeing 7 PSUM banks.
            # Two leftover group-2 fulls plus six group-3 dc=0..2 partial
            # accumulations run while the final norm chain executes, keeping
            # the PE busy and the HAM clock released; after the norm each
            # partial needs only its single dc=3 matmul. ----
            with tc.tile_pool(name="ps_y4", bufs=7, space="PSUM") as y4_ps:
                lead = []
                while proj_queue and len(lead) < 2:
                    lead.append(proj_queue.pop(0))
                for t, eh in lead:
                    emit_proj_half(t, eh, y4_ps, tag="y4")
                PART = [(12, 0), (12, 1), (13, 0), (13, 1), (14, 0), (14, 1)]
                parts = []
                for t, eh in PART:
                    yp = y4_ps.tile([128, 512], FP32, tag="y4",
                                    name=f"ypre{t}_{eh}")
                    for dc in range(3):
                        i = dc * 2 + eh
                        nc.tensor.matmul(yp, ot[dc][:, 128 * t:128 * t + 128],
                                         wo_all[:, 512 * i:512 * i + 512],
                                         start=(dc == 0), stop=False)
                    parts.append((t, eh, yp))
                nh_mul = pending_norm[0]()   # recips/casts + rsb matmuls
                pending_norm[0] = None
                nh_mul()                     # final ot write; queues group 3
                # finish partials in (eh=0, eh=1) pairs: both casts land in
                # one [128, 1024] tile so the drain DMA gets 2KB lines
                for t, eh, yp in parts:
                    i = 3 * 2 + eh
                    nc.tensor.matmul(yp, ot[3][:, 128 * t:128 * t + 128],
                                     wo_all[:, 512 * i:512 * i + 512],
                                     start=False, stop=True)
                for k4 in range(0, len(parts), 2):
                    (t, _, ypA), (_, _, ypB) = parts[k4], parts[k4 + 1]
                    ytp = ytp_pool.tile([128, 1024], BF16, tag="ytp")
                    nc.vector.tensor_copy(out=ytp[:, 0:512], in_=ypA)
                    nc.scalar.copy(out=ytp[:, 512:1024], in_=ypB)
                    nc.sync.dma_start(out=y_d[t], in_=ytp)
                rest = [x for x in proj_queue if x not in PART]
                for k4 in range(0, len(rest), 2):
                    if (k4 + 1 < len(rest) and rest[k4][0] == rest[k4 + 1][0]
                            and rest[k4][1] == 0):
                        t = rest[k4][0]
                        yps = []
                        for eh in (0, 1):
                            yp = y4_ps.tile([128, 512], FP32, tag="y4")
                            for dc in range(4):
                                i = dc * 2 + eh
                                nc.tensor.matmul(
                                    yp, ot[dc][:, 128 * t:128 * t + 128],
                                    wo_all[:, 512 * i:512 * i + 512],
                                    start=(dc == 0), stop=(dc == 3))
                            yps.append(yp)
                        ytp = ytp_pool.tile([128, 1024], BF16, tag="ytp")
                        nc.vector.tensor_copy(out=ytp[:, 0:512], in_=yps[0])
                        nc.scalar.copy(out=ytp[:, 512:1024], in_=yps[1])
                        nc.sync.dma_start(out=y_d[t], in_=ytp)
                    else:
                        emit_proj_half(*rest[k4], y4_ps, tag="y4")
                        if k4 + 1 < len(rest):
                            emit_proj_half(*rest[k4 + 1], y4_ps, tag="y4",
                                           cast_eng=nc.scalar.copy)
    nc.compile()
    return nc


def _get_nc():
    global _built
    if _built is None:
        _built = _build()
    return _built


def _in_maps(x, W, Wo):
    x = np.asarray(x, np.float32)
    W = np.asarray(W, np.float32)
    Wo = np.asarray(Wo, np.float32)

    cos2, sin2 = _rope_tables()
    tri = np.zeros((128, 128), np.float32)
    p_idx = np.arange(128)
    tri[p_idx[:, None] <= p_idx[None, :]] = 1.0  # valid: k <= q
    tri = tri.astype(BF)

    in_maps = []
    for core in range(NCORES):
        b, g = core // 2, core % 2
        xt = np.ascontiguousarray(x[b].T).astype(BF)                 # [D, L]
        # -> [l4, 128(d-part), 8(dchunk) x 512] contiguous
        xt = xt.reshape(8, 128, 4, 512).transpose(2, 1, 0, 3)
        xt = np.ascontiguousarray(xt).reshape(4, 128, 4096)
        wq = W[512 * g:512 * g + 512]                                # [512, D]
        wk = W[D + 512 * g:D + 512 * g + 512]
        wv = W[2 * D + 512 * g:2 * D + 512 * g + 512]
        wqk_t = np.ascontiguousarray(
            np.concatenate([wq, wk], 0).T).astype(BF)                # [D, 1024]
        # -> [echunk, 128(d-part), 1024(8 dchunk x 128 e)]
        wqk_t = wqk_t.reshape(8, 128, 8, 128).transpose(2, 1, 0, 3)
        wqk_t = np.ascontiguousarray(wqk_t.reshape(8, 128, 1024))
        wv_t = np.ascontiguousarray(wv.T).astype(BF)                 # [D, 512]
        wv_t = np.ascontiguousarray(
            wv_t.reshape(8, 128, 512).transpose(1, 0, 2))            # [128,8,512]
        wv_t = wv_t.reshape(128, 4096)
        wo_t = np.ascontiguousarray(Wo[:, 512 * g:512 * g + 512].T).astype(BF)
        wo_t = wo_t.reshape(4, 128, 2, 512).transpose(1, 0, 2, 3)    # [128,4,2,512]
        wo_t = np.ascontiguousarray(wo_t).reshape(128, 4096)
        in_maps.append({
            "xt": xt, "wqk": wqk_t, "wv": wv_t, "wo": wo_t,
            "cos2": cos2, "sin2": sin2, "trimask": tri,
        })
    return in_maps


def kernel(x, W, Wo):
    from concourse.bass_utils import run_bass_kernel_spmd

    res = run_bass_kernel_spmd(_get_nc(), _in_maps(x, W, Wo),
                               list(range(NCORES)))
    out = np.empty((B, L, D), np.float32)
    for b in range(B):
        yb = (res.results[2 * b]["y"].astype(np.float32)
              + res.results[2 * b + 1]["y"].astype(np.float32))
        out[b] = yb.reshape(L, D)
    return out


def _install_ntff_hook_shim():
    """The trimmed repo lacks antenv.axon_hooks; reconstruct it so
    run_bass_kernel_spmd(trace=True) can NTFF-profile through axon."""
    import sys as _sys, types
    if "antenv.axon_hooks" in _sys.modules:
        return
    import antenv  # noqa: F401
    from trn_agent_boot.trn_boot import _ntff_profile_via_ctypes
    hook = _ntff_profile_via_ctypes("/opt/axon/libaxon_pjrt.so")
    mod = types.ModuleType("antenv.axon_hooks")
    mod.set_axon_ntff_profile_hook = lambda h: None
    mod.get_axon_ntff_profile_hook = lambda: hook
    _sys.modules["antenv.axon_hooks"] = mod


def kernel_traced(x, W, Wo, tmpdir=None):
    """Run with NTFF tracing; returns exec time ns (trace in tmpdir)."""
    from concourse.bass_utils import run_bass_kernel_spmd

    _install_ntff_hook_shim()
    res = run_bass_kernel_spmd(_get_nc(), _in_maps(x, W, Wo),
                               list(range(NCORES)), trace=True, tmpdir=tmpdir)
    return res.exec_time_ns
